# revision 36
# baseline (speedup 1.0000x reference)
"""DND retrieval (episodic memory read) kernel for 8 Trainium2 NeuronCores.

Data-parallel over batch B=64 -> 8 envs per core, with step-aware
packing: only ceil(step/128) l-chunks per env are ever touched (the
rest are masked to zero by the softmax validity mask), so the host
packs exactly those chunks, assigns envs to cores by sorted rank so
every core shares one compiled chunk pattern C*, and the kernel skips
the dead ~45% of keys/vals DMA and PE work.

Precision: keys (with rpe * 64/sqrt(K) folded in) and the q-side MLP
stream as fp8e4m3 (weights x32, qc x32, q x16 host/chip scales); the
scores and Wq matmuls run in fp8 DoubleRow mode (2 contraction rows
per partition, 2x PE rate). vals and output-side weights stay bf16
(fp8 there pushes error past budget).

Scores are processed in 512-column windows of the packed image through
a 2-bank PSUM ring: scores -> exp(S/1024) -> multiply by a precomputed
validity mask -> unnormalized probs transpose straight into the value
matmul; softmax 1/Z is applied to the [64, 512] result instead
(linearity), so nothing waits on the global sum. Scores are tiny
(|s| < 0.3), so no max pass is needed.
"""
from contextlib import ExitStack

import numpy as np
import ml_dtypes

import concourse.bass as bass
import concourse.tile as tile
from concourse import bacc, mybir
from concourse.bass_utils import run_bass_kernel_spmd
from concourse.masks import make_identity

F32 = mybir.dt.float32
BF16 = mybir.dt.bfloat16
FP8 = mybir.dt.float8e4
AF = mybir.ActivationFunctionType
OP = mybir.AluOpType
DR = mybir.MatmulPerfMode.DoubleRow

L = 1024
B = 64        # rows of the batched softmax image: (slot, head)
BL = 8        # envs (slots) per core
KD = 512
VD = 512
H = 8
MEMB = 256
SDIM = 512
HID = 512
RIMQ = 512
LAT = KD - MEMB
NCORES = 8
KC = KD // 128
RSQK = 1.0 / np.sqrt(np.float32(KD))
KSCALE = 64.0          # folded into keys on host
WSCALE = 32.0          # fp8 weight scale
QCS = 32.0             # qc activation fp8 scale
QS = 16.0              # q fp8 scale inside Qpad
NBF16 = np.dtype(ml_dtypes.bfloat16)
NFP8 = np.dtype(ml_dtypes.float8_e4m3)
SEQ = [0, 7, 1, 6, 2, 5, 3, 4]   # packed slot order

_CACHE: dict = {}


def _emit(nc: bass.Bass, tc: tile.TileContext, ctx: ExitStack, io: dict,
          cstar: tuple):
    # ---- packed geometry (compile-time) ----
    seqc = [cstar[s] for s in SEQ]
    offs = np.concatenate([[0], np.cumsum(seqc)])
    NCH = int(offs[-1])
    W = NCH * 128
    owner = []                       # chunk idx -> slot
    for p, s in enumerate(SEQ):
        owner += [s] * seqc[p]
    NW = (NCH + 3) // 4              # 512-col score windows
    NS = (NW + 1) // 2               # keys DMA slabs (2 windows each)

    pool = ctx.enter_context(tc.tile_pool(name="main", bufs=1))
    kpool = ctx.enter_context(tc.tile_pool(name="keys", bufs=2 * NS))
    wpool = ctx.enter_context(tc.tile_pool(name="wstream", bufs=2))
    psum = ctx.enter_context(tc.tile_pool(name="ps", bufs=3, space="PSUM"))
    spsum = ctx.enter_context(tc.tile_pool(name="ps2", bufs=2, space="PSUM"))
    rpsum = ctx.enter_context(tc.tile_pool(name="ps3", bufs=1, space="PSUM"))

    identb = pool.tile([128, 128], BF16)
    make_identity(nc, identb[:])

    def bias_tile(name, nch, eng=None):
        t = pool.tile([128, nch], F32, tag="b" + name)
        (eng or nc.sync).dma_start(t[:], io[name][:])
        return t

    # host-built per-partition exp bias: 0 where l valid for the chunk's
    # owner env, -1e30 otherwise (masks fold into the exp activation)
    rowbias = pool.tile([128, NCH], F32)
    nc.sync.dma_start(rowbias[:], io["rowbias"][:])

    # ---------------- Phase A: q-side MLP (fp8, DoubleRow Wq) -------------
    stateT_n = pool.tile([128, SDIM // 128, BL], FP8)
    nc.sync.dma_start(stateT_n[:], io["stateT"][:])
    latT_n = pool.tile([128, LAT // 128, BL], BF16)
    nc.sync.dma_start(latT_n[:], io["latT"][:])

    bst = bias_tile("b_state", 2)        # x32
    bcq1 = bias_tile("bcq1", 4)          # x32
    bcq2 = bias_tile("bcq2", 4)          # x32
    bq = bias_tile("bq", 32)             # x(32*QCS)

    stateT = [stateT_n[:, c, :] for c in range(SDIM // 128)]
    latT = [latT_n[:, c, :] for c in range(LAT // 128)]

    def layer_T(xT_chunks, w_name, b_tile, n_out, tag, wdt=BF16, scale=None,
                out_dt=BF16, eng=None):
        nk = len(xT_chunks)
        w = wpool.tile([128, nk, n_out], wdt,
                       tag="Wstg8" if wdt == FP8 else "Wstgb")
        (eng or nc.sync).dma_start(w[:], io[w_name][:])
        outs = []
        for j in range(n_out // 128):
            ps = psum.tile([128, BL], F32, tag="sm")
            for k in range(nk):
                nc.tensor.matmul(ps[:], w[:, k, j * 128:(j + 1) * 128],
                                 xT_chunks[k], start=(k == 0),
                                 stop=(k == nk - 1), skip_group_check=True)
            t = pool.tile([128, BL], out_dt, tag=f"{tag}{j}")
            if scale is None:
                nc.vector.tensor_scalar(out=t[:], in0=ps[:],
                                        scalar1=b_tile[:, j:j + 1],
                                        scalar2=None, op0=OP.add)
            else:
                nc.vector.tensor_scalar(out=t[:], in0=ps[:],
                                        scalar1=b_tile[:, j:j + 1],
                                        scalar2=scale, op0=OP.add,
                                        op1=OP.mult)
            outs.append(t[:])
        return outs

    RW = 1.0 / WSCALE
    xT = layer_T(stateT, "W_state", bst, MEMB, "xT", wdt=FP8, scale=RW) + latT
    h1T = layer_T(xT, "Wcq1", bcq1, HID, "h1", wdt=FP8, scale=RW,
                  eng=nc.scalar)
    # qc layer -> single fp8 tile (x QCS), consumed as DoubleRow lhsT.
    # Padded to QCW columns: dual-fp8 LDWEIGHTS rejects 8-wide loads.
    QCW = 32
    qcT = pool.tile([128, KC, QCW], FP8)
    nc.gpsimd.memset(qcT[:], 0.0)
    wcq2 = wpool.tile([128, KC, KD], FP8, tag="Wstg8")
    nc.sync.dma_start(wcq2[:], io["Wcq2"][:])
    for j in range(KC):
        ps = psum.tile([128, BL], F32, tag="sm")
        for k in range(KC):
            nc.tensor.matmul(ps[:], wcq2[:, k, j * 128:(j + 1) * 128],
                             h1T[k], start=(k == 0), stop=(k == KC - 1),
                             skip_group_check=True)
        nc.vector.tensor_scalar(out=qcT[:, j, 0:BL], in0=ps[:],
                                scalar1=bcq2[:, j:j + 1], scalar2=QCS / 32.0,
                                op0=OP.add, op1=OP.mult)

    # Wq in DoubleRow fp8: out [8, 512] per (jg, kcp), then transpose and
    # scatter into Qpad (fp8, xQS) diagonal windows.
    Qpad = pool.tile([128, 2, 2, BL, 72], FP8)
    nc.gpsimd.memset(Qpad[:], 0.0)
    wq = pool.tile([128, 2, 2, H * KD], FP8)
    for kcp in range(2):
        (nc.sync if kcp == 0 else nc.scalar).dma_start(
            wq[:, kcp, :, :], io["Wq"][:, kcp, :, :])
    QSC = QS / (32.0 * QCS)
    for jg in range(8):
        ps = spsum.tile([QCW, 512], F32, tag="sp")
        for kcp in range(2):
            nc.tensor.matmul(ps[:], qcT[:, 2 * kcp:2 * kcp + 2, :],
                             wq[:, kcp, :, jg * 512:(jg + 1) * 512],
                             start=(kcp == 0), stop=(kcp == 1),
                             perf_mode=DR, skip_group_check=True)
        qsb = pool.tile([BL, 512], BF16, tag="qsb")
        nc.scalar.copy(qsb[:], ps[0:BL, :])
        for jj in range(4):
            j = jg * 4 + jj
            h, kc = j // KC, j % KC
            tp = psum.tile([128, BL], BF16, tag="sm")
            nc.tensor.transpose(tp[:], qsb[:, jj * 128:(jj + 1) * 128],
                                identb[0:BL, 0:BL])
            nc.vector.tensor_scalar(
                out=Qpad[:, kc // 2, kc % 2, :, h], in0=tp[:],
                scalar1=bq[:, j:j + 1], scalar2=QSC, op0=OP.add, op1=OP.mult)

    # ------- keys + vals interleaved per slab (pipeline chases the stream) --
    slabs = []
    vres = pool.tile([128, NCH, VD], BF16)
    vengs = [nc.gpsimd, nc.sync, nc.scalar]
    for si in range(NS):
        c0, c1 = 8 * si, min(8 * si + 8, NCH)
        kts = []
        for kcp in range(2):
            kt = kpool.tile([128, 2, 1024], FP8, tag="kt")
            (nc.sync if kcp == 0 else nc.scalar).dma_start(
                kt[:, :, 0:(c1 - c0) * 128],
                io["keysT"][:, kcp, :, c0 * 128:c1 * 128])
            kts.append(kt)
        slabs.append(kts)
        vengs[si % 3].dma_start(vres[:, c0:c1, :], io["vals"][:, c0:c1, :])
    wagg = pool.tile([128, 32, VD], BF16)
    waeng = [nc.sync, nc.gpsimd, nc.scalar, nc.gpsimd]
    for gi in range(4):
        waeng[gi].dma_start(wagg[:, gi * 8:(gi + 1) * 8, :],
                            io["Wagg"][:, gi * 8:(gi + 1) * 8, :])

    # ------- per-chunk: scoresT -> exp(+bias) -> value matmul + Z ----------
    # scoresT [128(l), 8(h)] per chunk; exp writes masked unnormalized probs
    # straight into the transposed EVT image the value matmul consumes.
    EVT = pool.tile([128, NCH, B], BF16)
    nc.gpsimd.memset(EVT[:], 0.0)
    onesb = pool.tile([128, 1], BF16)
    nc.gpsimd.memset(onesb[:], 1.0)
    rps = rpsum.tile([B, VD], F32, tag="rp")
    zps = rpsum.tile([1, B], F32, tag="z")

    def chunkwork(i):
        nc.tensor.matmul(rps[:], EVT[:, i, :], vres[:, i, :],
                         start=(i == 0), stop=(i == NCH - 1),
                         skip_group_check=True)
        nc.tensor.matmul(zps[:], onesb[:], EVT[:, i, :],
                         start=(i == 0), stop=(i == NCH - 1),
                         skip_group_check=True)

    for i in range(NCH):
        s = owner[i]
        si, sc0 = i // 8, (i % 8) * 128
        kts = slabs[si]
        sgt = psum.tile([128, H], F32, tag="sm")
        for kcp in range(2):
            nc.tensor.matmul(sgt[:], kts[kcp][:, :, sc0:sc0 + 128],
                             Qpad[:, kcp, :, s, 0:H],
                             start=(kcp == 0), stop=(kcp == 1),
                             perf_mode=DR, skip_group_check=True)
        nc.scalar.activation(EVT[:, i, s * H:(s + 1) * H], sgt[:], AF.Exp,
                             bias=rowbias[:, i:i + 1],
                             scale=1.0 / (KSCALE * QS))
        if i > 1:
            chunkwork(i - 2)
    for i in range(max(NCH - 2, 0), NCH):
        chunkwork(i)

    # Z -> [64, 1] -> R = 1/Z folded into the result readout
    zsb = pool.tile([1, B], BF16)
    nc.vector.tensor_copy(zsb[:], zps[:])
    ztp = psum.tile([B, 1], BF16, tag="sm")
    nc.tensor.transpose(ztp[:], zsb[:], identb[0:1, 0:1])
    R = pool.tile([B, 1], F32)
    nc.vector.reciprocal(R[:], ztp[:])
    rsb = pool.tile([B, VD], BF16, tag="rs")
    nc.vector.tensor_scalar(out=rsb[:], in0=rps[:], scalar1=R[:, 0:1],
                            scalar2=None, op0=OP.mult)
    RT = pool.tile([128, VD // 128, B], BF16)
    for vc in range(VD // 128):
        tr = psum.tile([128, B], BF16, tag="sm")
        nc.tensor.transpose(tr[:], rsb[:, vc * 128:(vc + 1) * 128],
                            identb[0:B, 0:B])
        nc.vector.tensor_copy(RT[:, vc, :], tr[:])

    # ---------------- Phase E: output MLP chain (bf16) ---------------------
    bagg = bias_tile("bagg", 4)
    brk1 = bias_tile("brk1", 4)
    brv1 = bias_tile("brv1", 4, eng=nc.scalar)

    aggp = spsum.tile([BL, VD], F32, tag="sp")
    for c in range(32):
        h, vc = c // 4, c % 4
        nc.tensor.matmul(aggp[:], RT[:, vc, h:B:H], wagg[:, c, :],
                         start=(c == 0), stop=(c == 31),
                         skip_group_check=True)
    aggsb = pool.tile([BL, VD], BF16, tag="aggsb")
    nc.scalar.copy(aggsb[:], aggp[:])
    AT = []
    for j in range(VD // 128):
        tp = psum.tile([128, BL], BF16, tag="sm")
        nc.tensor.transpose(tp[:], aggsb[:, j * 128:(j + 1) * 128],
                            identb[0:BL, 0:BL])
        t = pool.tile([128, BL], BF16, tag=f"AT{j}")
        nc.vector.tensor_scalar(out=t[:], in0=tp[:],
                                scalar1=bagg[:, j:j + 1],
                                scalar2=None, op0=OP.add)
        AT.append(t[:])

    ones = pool.tile([1, BL], F32)
    nc.gpsimd.memset(ones[:], 1.0)

    def bias_bcast(name, eng=None):
        brow = pool.tile([1, 512], F32, tag="br" + name)
        (eng or nc.sync).dma_start(brow[:], io[name][:])
        bb = spsum.tile([BL, 512], F32, tag="sp")
        nc.tensor.matmul(bb[:], ones[:], brow[:], start=True, stop=True)
        bsb = pool.tile([BL, 512], F32, tag="bs" + name)
        nc.vector.tensor_copy(bsb[:], bb[:])
        return bsb

    bk2 = bias_bcast("brk2_flat")
    bv2 = bias_bcast("brv2_flat", eng=nc.scalar)

    def layer_nat(xT_chunks, w_name, n_out, eng=None):
        nk = len(xT_chunks)
        w = wpool.tile([128, nk, n_out], BF16, tag="Wstgb")
        (eng or nc.sync).dma_start(w[:], io[w_name][:])
        ps = spsum.tile([BL, n_out], F32, tag="sp")
        for k in range(nk):
            nc.tensor.matmul(ps[:], xT_chunks[k], w[:, k, :],
                             start=(k == 0), stop=(k == nk - 1),
                             skip_group_check=True)
        return ps

    hkT = layer_T(AT, "Wrk1", brk1, HID, "hk")
    ok_ps = layer_nat(hkT, "Wrk2", RIMQ)
    hvT = layer_T(AT, "Wrv1", brv1, HID, "hv", eng=nc.scalar)
    ov_ps = layer_nat(hvT, "Wrv2", VD, eng=nc.scalar)

    for name, ps_, bias_sb in (("out_key", ok_ps, bk2), ("out_val", ov_ps, bv2)):
        onat = pool.tile([BL, 512], F32, tag="o" + name)
        nc.vector.tensor_tensor(out=onat[:], in0=ps_[:], in1=bias_sb[:],
                                op=OP.add)
        nc.sync.dma_start(io[name][:], onat[:])


def _build(cstar):
    seqc = [cstar[s] for s in SEQ]
    NCH = int(sum(seqc))
    W = NCH * 128
    NW = (NCH + 3) // 4
    nc = bacc.Bacc("TRN2", target_bir_lowering=False, debug=False,
                   num_devices=NCORES)
    io = {}

    def din(name, shape, dt=BF16):
        io[name] = nc.dram_tensor(name, shape, dt, kind="ExternalInput").ap()

    din("keysT", [128, 2, 2, W], FP8)
    din("vals", [128, NCH, VD])
    din("rowbias", [128, NCH], F32)
    din("stateT", [128, SDIM // 128, BL], FP8)
    din("latT", [128, LAT // 128, BL])
    din("W_state", [128, KC, MEMB], FP8)
    din("b_state", [128, 2], F32)
    din("Wcq1", [128, KC, HID], FP8)
    din("bcq1", [128, 4], F32)
    din("Wcq2", [128, KC, KD], FP8)
    din("bcq2", [128, 4], F32)
    din("Wq", [128, 2, 2, H * KD], FP8)
    din("bq", [128, 32], F32)
    din("Wagg", [128, 32, VD])
    din("bagg", [128, 4], F32)
    din("Wrk1", [128, KC, HID])
    din("brk1", [128, 4], F32)
    din("Wrk2", [128, KC, RIMQ])
    din("brk2_flat", [1, 512], F32)
    din("Wrv1", [128, KC, HID])
    din("brv1", [128, 4], F32)
    din("Wrv2", [128, KC, VD])
    din("brv2_flat", [1, 512], F32)
    io["out_key"] = nc.dram_tensor("out_key", [BL, RIMQ], F32,
                                   kind="ExternalOutput").ap()
    io["out_val"] = nc.dram_tensor("out_val", [BL, VD], F32,
                                   kind="ExternalOutput").ap()

    with tile.TileContext(nc) as tc, ExitStack() as ctx:
        _emit(nc, tc, ctx, io, cstar)
    nc.compile()
    return nc


def _rsb(bias, nch, scale=1.0):
    return np.ascontiguousarray(
        np.asarray(bias, np.float32).reshape(nch, 128).T * scale)


def _wchunk(w, dt=NBF16, scale=1.0):
    w = np.asarray(w, np.float32) * scale
    f, c = w.shape
    return np.ascontiguousarray(
        w.reshape(f // 128, 128, c).transpose(1, 0, 2)).astype(dt)


def _actT(x, dt):
    x = np.asarray(x, np.float32)
    bl, f = x.shape
    return np.ascontiguousarray(
        x.T.reshape(f // 128, 128, bl).transpose(1, 0, 2)).astype(dt)


def _plan(step):
    cb = np.clip((np.asarray(step, np.int64) + 127) // 128, 1, 8)
    order = np.argsort(-cb, kind="stable")
    cstar = tuple(int(cb[order[8 * s]]) for s in range(BL))
    return order, cstar


def _shard(inputs):
    f = lambda x: np.asarray(x, np.float32)
    keys, vals, rpe = f(inputs["keys"]), f(inputs["vals"]), f(inputs["rpe_mod"])
    step = np.asarray(inputs["step"]).astype(np.int64)
    state, lat = f(inputs["state"]), f(inputs["task_inference_latent"])

    order, cstar = _plan(step)
    seqc = [cstar[s] for s in SEQ]
    offs = np.concatenate([[0], np.cumsum(seqc)])
    NCH = int(offs[-1])
    NW = (NCH + 3) // 4

    shared = {
        "W_state": _wchunk(inputs["W_state"], NFP8, WSCALE),
        "b_state": _rsb(inputs["b_state"], 2, WSCALE),
        "Wcq1": _wchunk(inputs["Wcq1"], NFP8, WSCALE),
        "bcq1": _rsb(inputs["bcq1"], 4, WSCALE),
        "Wcq2": _wchunk(inputs["Wcq2"], NFP8, WSCALE),
        "bcq2": _rsb(inputs["bcq2"], 4, WSCALE),
        "Wq": _wchunk(inputs["Wq"], NFP8, WSCALE).reshape(128, 2, 2, H * KD),
        "bq": _rsb(inputs["bq"], 32, WSCALE * QCS),
        "Wagg": _wchunk(inputs["Wagg"]),
        "bagg": _rsb(inputs["bagg"], 4),
        "Wrk1": _wchunk(inputs["Wrk1"]), "brk1": _rsb(inputs["brk1"], 4),
        "Wrk2": _wchunk(inputs["Wrk2"]),
        "brk2_flat": np.ascontiguousarray(f(inputs["brk2"])[None, :]),
        "Wrv1": _wchunk(inputs["Wrv1"]), "brv1": _rsb(inputs["brv1"], 4),
        "Wrv2": _wchunk(inputs["Wrv2"]),
        "brv2_flat": np.ascontiguousarray(f(inputs["brv2"])[None, :]),
    }
    kfold = keys * rpe * (KSCALE * RSQK)            # [L, 64, K]
    in_maps = []
    for m in range(NCORES):
        envs = [int(order[8 * s + m]) for s in range(BL)]
        kp = np.zeros((128, 2, 2, NCH * 128), NFP8)
        vp = np.zeros((128, NCH, VD), NBF16)
        rowbias = np.zeros((128, NCH), np.float32)
        for p, s in enumerate(SEQ):
            e = envs[s]
            nl = cstar[s] * 128
            c0, c1 = int(offs[p]), int(offs[p + 1])
            kb = kfold[:nl, e, :].T.reshape(2, 2, 128, nl).transpose(
                2, 0, 1, 3)
            kp[:, :, :, c0 * 128:c1 * 128] = kb.astype(NFP8)
            vb = vals[:nl, e, :].reshape(cstar[s], 128, VD).transpose(1, 0, 2)
            vp[:, c0:c1, :] = vb.astype(NBF16)
            labs = (np.arange(128)[:, None]
                    + 128 * np.arange(c1 - c0)[None, :])
            rowbias[:, c0:c1] = np.where(labs < int(step[e]), 0.0, -1e30)
        in_maps.append({
            "keysT": kp, "vals": vp, "rowbias": rowbias,
            "stateT": _actT(state[envs], NFP8),
            "latT": _actT(lat[envs], NBF16),
            **shared,
        })
    return in_maps, order


def kernel(**inputs):
    order, cstar = _plan(inputs["step"])
    nc = _CACHE.get(cstar)
    if nc is None:
        nc = _CACHE[cstar] = _build(cstar)
    in_maps, order = _shard(inputs)
    res = run_bass_kernel_spmd(nc, in_maps, list(range(NCORES)),
                               **_CACHE.get("run_kwargs", {}))
    _CACHE["last_result"] = res
    ok = np.empty((B, RIMQ), np.float32)
    ov = np.empty((B, VD), np.float32)
    for m in range(NCORES):
        for s in range(BL):
            e = int(order[8 * s + m])
            ok[e] = res.results[m]["out_key"][s]
            ov[e] = res.results[m]["out_val"][s]
    return ok[:, None, :], ov[:, None, :]


# revision 39
# speedup vs baseline: 1.0353x; 1.0353x over previous
"""DND retrieval (episodic memory read) kernel for 8 Trainium2 NeuronCores.

Data-parallel over batch B=64 -> 8 envs per core, with step-aware
packing: only ceil(step/128) l-chunks per env are ever touched (the
rest are masked to zero by the softmax validity mask), so the host
packs exactly those chunks, assigns envs to cores by sorted rank so
every core shares one compiled chunk pattern C*, and the kernel skips
the dead ~45% of keys/vals DMA and PE work.

Precision: keys (with rpe * 64/sqrt(K) folded in) and the q-side MLP
stream as fp8e4m3 (weights x32, qc x32, q x16 host/chip scales); the
scores and Wq matmuls run in fp8 DoubleRow mode (2 contraction rows
per partition, 2x PE rate). vals and output-side weights stay bf16
(fp8 there pushes error past budget).

Scores are processed in 512-column windows of the packed image through
a 2-bank PSUM ring: scores -> exp(S/1024) -> multiply by a precomputed
validity mask -> unnormalized probs transpose straight into the value
matmul; softmax 1/Z is applied to the [64, 512] result instead
(linearity), so nothing waits on the global sum. Scores are tiny
(|s| < 0.3), so no max pass is needed.
"""
from contextlib import ExitStack

import numpy as np
import ml_dtypes

import concourse.bass as bass
import concourse.tile as tile
from concourse import bacc, mybir
from concourse.bass_utils import run_bass_kernel_spmd
from concourse.masks import make_identity

F32 = mybir.dt.float32
BF16 = mybir.dt.bfloat16
FP8 = mybir.dt.float8e4
AF = mybir.ActivationFunctionType
OP = mybir.AluOpType
DR = mybir.MatmulPerfMode.DoubleRow

L = 1024
B = 64        # rows of the batched softmax image: (slot, head)
BL = 8        # envs (slots) per core
KD = 512
VD = 512
H = 8
MEMB = 256
SDIM = 512
HID = 512
RIMQ = 512
LAT = KD - MEMB
NCORES = 8
KC = KD // 128
RSQK = 1.0 / np.sqrt(np.float32(KD))
KSCALE = 64.0          # folded into keys on host
WSCALE = 32.0          # fp8 weight scale
QCS = 32.0             # qc activation fp8 scale
QS = 16.0              # q fp8 scale inside Qpad
NBF16 = np.dtype(ml_dtypes.bfloat16)
NFP8 = np.dtype(ml_dtypes.float8_e4m3)
SEQ = [0, 7, 1, 6, 2, 5, 3, 4]   # packed slot order

_CACHE: dict = {}


def _emit(nc: bass.Bass, tc: tile.TileContext, ctx: ExitStack, io: dict,
          cstar: tuple):
    # ---- packed geometry (compile-time) ----
    seqc = [cstar[s] for s in SEQ]
    offs = np.concatenate([[0], np.cumsum(seqc)])
    NCH = int(offs[-1])
    W = NCH * 128
    owner = []                       # chunk idx -> slot
    for p, s in enumerate(SEQ):
        owner += [s] * seqc[p]
    NW = (NCH + 3) // 4              # 512-col score windows
    NS = (NW + 1) // 2               # keys DMA slabs (2 windows each)

    pool = ctx.enter_context(tc.tile_pool(name="main", bufs=1))
    kpool = ctx.enter_context(tc.tile_pool(name="keys", bufs=2 * NS))
    wpool = ctx.enter_context(tc.tile_pool(name="wstream", bufs=2))
    psum = ctx.enter_context(tc.tile_pool(name="ps", bufs=3, space="PSUM"))
    spsum = ctx.enter_context(tc.tile_pool(name="ps2", bufs=2, space="PSUM"))
    rpsum = ctx.enter_context(tc.tile_pool(name="ps3", bufs=1, space="PSUM"))

    identb = pool.tile([128, 128], BF16)
    make_identity(nc, identb[:])

    def bias_tile(name, nch, eng=None):
        t = pool.tile([128, nch], F32, tag="b" + name)
        (eng or nc.sync).dma_start(t[:], io[name][:])
        return t

    # One fp8 blob (5 KB lines) carries all phase-A operands: the former
    # per-tensor DMAs had 8-140 B partition lines whose descriptor overhead
    # stalled the sync queue ~25 us before W_state even started.
    A8 = pool.tile([128, 5168], FP8)
    nc.sync.dma_start(A8[:], io["A8"][:])
    CF = pool.tile([128, NCH + 54], F32)
    nc.sync.dma_start(CF[:], io["CF"][:])
    rowbias = CF[:, 0:NCH]
    bst = CF[:, NCH:NCH + 2]
    bcq1 = CF[:, NCH + 2:NCH + 6]
    bcq2 = CF[:, NCH + 6:NCH + 10]
    bq = CF[:, NCH + 10:NCH + 42]

    # ---------------- Phase A: q-side MLP (fp8, DoubleRow Wq) -------------
    stateT_n = A8[:, 0:32].rearrange("p (k b) -> p k b", k=4)
    latT_n = A8[:, 32:48].rearrange("p (k b) -> p k b", k=2)
    w_state = A8[:, 48:1072].rearrange("p (k c) -> p k c", k=4)
    w_cq1 = A8[:, 1072:3120].rearrange("p (k c) -> p k c", k=4)
    w_cq2 = A8[:, 3120:5168].rearrange("p (k c) -> p k c", k=4)

    stateT = [stateT_n[:, c, :] for c in range(SDIM // 128)]
    latT = [latT_n[:, c, :] for c in range(LAT // 128)]

    def layer_T(xT_chunks, w, b_tile, n_out, tag, scale=None):
        nk = len(xT_chunks)
        outs = []
        for j in range(n_out // 128):
            ps = psum.tile([128, BL], F32, tag="sm")
            for k in range(nk):
                nc.tensor.matmul(ps[:], w[:, k, j * 128:(j + 1) * 128],
                                 xT_chunks[k], start=(k == 0),
                                 stop=(k == nk - 1), skip_group_check=True)
            t = pool.tile([128, BL], BF16, tag=f"{tag}{j}")
            if scale is None:
                nc.vector.tensor_scalar(out=t[:], in0=ps[:],
                                        scalar1=b_tile[:, j:j + 1],
                                        scalar2=None, op0=OP.add)
            else:
                nc.vector.tensor_scalar(out=t[:], in0=ps[:],
                                        scalar1=b_tile[:, j:j + 1],
                                        scalar2=scale, op0=OP.add,
                                        op1=OP.mult)
            outs.append(t[:])
        return outs

    def layer_Tio(xT_chunks, w_name, b_tile, n_out, tag, eng=None):
        nk = len(xT_chunks)
        w = wpool.tile([128, nk, n_out], BF16, tag="Wstgb")
        (eng or nc.sync).dma_start(w[:], io[w_name][:])
        return layer_T(xT_chunks, w[:], b_tile, n_out, tag)

    RW = 1.0 / WSCALE
    xT = layer_T(stateT, w_state, bst, MEMB, "xT", scale=RW) + latT
    h1T = layer_T(xT, w_cq1, bcq1, HID, "h1", scale=RW)
    # qc layer -> single fp8 tile (x QCS), consumed as DoubleRow lhsT.
    # Padded to QCW columns: dual-fp8 LDWEIGHTS rejects 8-wide loads.
    QCW = 32
    qcT = pool.tile([128, KC, QCW], FP8)
    nc.gpsimd.memset(qcT[:], 0.0)
    for j in range(KC):
        ps = psum.tile([128, BL], F32, tag="sm")
        for k in range(KC):
            nc.tensor.matmul(ps[:], w_cq2[:, k, j * 128:(j + 1) * 128],
                             h1T[k], start=(k == 0), stop=(k == KC - 1),
                             skip_group_check=True)
        nc.vector.tensor_scalar(out=qcT[:, j, 0:BL], in0=ps[:],
                                scalar1=bcq2[:, j:j + 1], scalar2=QCS / 32.0,
                                op0=OP.add, op1=OP.mult)

    # Wq in DoubleRow fp8: out [8, 512] per (jg, kcp), then transpose and
    # scatter into Qpad (fp8, xQS) diagonal windows.
    Qpad = pool.tile([128, 2, 2, BL, 72], FP8)
    nc.gpsimd.memset(Qpad[:], 0.0)
    wq = pool.tile([128, 2, 2, H * KD], FP8)
    nc.scalar.dma_start(wq[:], io["Wq"][:])
    QSC = QS / (32.0 * QCS)
    for jg in range(8):
        ps = spsum.tile([QCW, 512], F32, tag="sp")
        for kcp in range(2):
            nc.tensor.matmul(ps[:], qcT[:, 2 * kcp:2 * kcp + 2, :],
                             wq[:, kcp, :, jg * 512:(jg + 1) * 512],
                             start=(kcp == 0), stop=(kcp == 1),
                             perf_mode=DR, skip_group_check=True)
        qsb = pool.tile([BL, 512], BF16, tag="qsb")
        nc.scalar.copy(qsb[:], ps[0:BL, :])
        for jj in range(4):
            j = jg * 4 + jj
            h, kc = j // KC, j % KC
            tp = psum.tile([128, BL], BF16, tag="sm")
            nc.tensor.transpose(tp[:], qsb[:, jj * 128:(jj + 1) * 128],
                                identb[0:BL, 0:BL])
            nc.vector.tensor_scalar(
                out=Qpad[:, kc // 2, kc % 2, :, h], in0=tp[:],
                scalar1=bq[:, j:j + 1], scalar2=QSC, op0=OP.add, op1=OP.mult)

    # ------- keys + vals interleaved per slab (pipeline chases the stream) --
    slabs = []
    vres = pool.tile([128, NCH, VD], BF16)
    vengs = [nc.gpsimd, nc.sync, nc.scalar]
    for si in range(NS):
        c0, c1 = 8 * si, min(8 * si + 8, NCH)
        kts = []
        for kcp in range(2):
            kt = kpool.tile([128, 2, 1024], FP8, tag="kt")
            (nc.sync if kcp == 0 else nc.scalar).dma_start(
                kt[:, :, 0:(c1 - c0) * 128],
                io["keysT"][:, kcp, :, c0 * 128:c1 * 128])
            kts.append(kt)
        slabs.append(kts)
        vengs[si % 3].dma_start(vres[:, c0:c1, :], io["vals"][:, c0:c1, :])
    wagg = pool.tile([128, 32, VD], BF16)
    waeng = [nc.sync, nc.gpsimd, nc.scalar, nc.gpsimd]
    for gi in range(4):
        waeng[gi].dma_start(wagg[:, gi * 8:(gi + 1) * 8, :],
                            io["Wagg"][:, gi * 8:(gi + 1) * 8, :])

    # ------- per-chunk: scoresT -> exp(+bias) -> value matmul + Z ----------
    # scoresT [128(l), 8(h)] per chunk; exp writes masked unnormalized probs
    # straight into the transposed EVT image the value matmul consumes.
    EVT = pool.tile([128, NCH, B], BF16)
    nc.gpsimd.memset(EVT[:], 0.0)
    onesb = pool.tile([128, 1], BF16)
    nc.gpsimd.memset(onesb[:], 1.0)
    rps = rpsum.tile([B, VD], F32, tag="rp")
    zps = rpsum.tile([1, B], F32, tag="z")

    def chunkwork(i):
        nc.tensor.matmul(rps[:], EVT[:, i, :], vres[:, i, :],
                         start=(i == 0), stop=(i == NCH - 1),
                         skip_group_check=True)
        nc.tensor.matmul(zps[:], onesb[:], EVT[:, i, :],
                         start=(i == 0), stop=(i == NCH - 1),
                         skip_group_check=True)

    for i in range(NCH):
        s = owner[i]
        si, sc0 = i // 8, (i % 8) * 128
        kts = slabs[si]
        sgt = psum.tile([128, H], F32, tag="sm")
        for kcp in range(2):
            nc.tensor.matmul(sgt[:], kts[kcp][:, :, sc0:sc0 + 128],
                             Qpad[:, kcp, :, s, 0:H],
                             start=(kcp == 0), stop=(kcp == 1),
                             perf_mode=DR, skip_group_check=True)
        nc.scalar.activation(EVT[:, i, s * H:(s + 1) * H], sgt[:], AF.Exp,
                             bias=rowbias[:, i:i + 1],
                             scale=1.0 / (KSCALE * QS))
        if i > 1:
            chunkwork(i - 2)
    for i in range(max(NCH - 2, 0), NCH):
        chunkwork(i)

    # Z -> [64, 1] -> R = 1/Z folded into the result readout
    zsb = pool.tile([1, B], BF16)
    nc.vector.tensor_copy(zsb[:], zps[:])
    ztp = psum.tile([B, 1], BF16, tag="sm")
    nc.tensor.transpose(ztp[:], zsb[:], identb[0:1, 0:1])
    R = pool.tile([B, 1], F32)
    nc.vector.reciprocal(R[:], ztp[:])
    rsb = pool.tile([B, VD], BF16, tag="rs")
    nc.vector.tensor_scalar(out=rsb[:], in0=rps[:], scalar1=R[:, 0:1],
                            scalar2=None, op0=OP.mult)
    RT = pool.tile([128, VD // 128, B], BF16)
    for vc in range(VD // 128):
        tr = psum.tile([128, B], BF16, tag="sm")
        nc.tensor.transpose(tr[:], rsb[:, vc * 128:(vc + 1) * 128],
                            identb[0:B, 0:B])
        nc.vector.tensor_copy(RT[:, vc, :], tr[:])

    # ---------------- Phase E: output MLP chain (bf16) ---------------------
    bagg = CF[:, NCH + 42:NCH + 46]
    brk1 = CF[:, NCH + 46:NCH + 50]
    brv1 = CF[:, NCH + 50:NCH + 54]

    aggp = spsum.tile([BL, VD], F32, tag="sp")
    for c in range(32):
        h, vc = c // 4, c % 4
        nc.tensor.matmul(aggp[:], RT[:, vc, h:B:H], wagg[:, c, :],
                         start=(c == 0), stop=(c == 31),
                         skip_group_check=True)
    aggsb = pool.tile([BL, VD], BF16, tag="aggsb")
    nc.scalar.copy(aggsb[:], aggp[:])
    AT = []
    for j in range(VD // 128):
        tp = psum.tile([128, BL], BF16, tag="sm")
        nc.tensor.transpose(tp[:], aggsb[:, j * 128:(j + 1) * 128],
                            identb[0:BL, 0:BL])
        t = pool.tile([128, BL], BF16, tag=f"AT{j}")
        nc.vector.tensor_scalar(out=t[:], in0=tp[:],
                                scalar1=bagg[:, j:j + 1],
                                scalar2=None, op0=OP.add)
        AT.append(t[:])

    ones = pool.tile([1, BL], F32)
    nc.gpsimd.memset(ones[:], 1.0)

    def bias_bcast(name, eng=None):
        brow = pool.tile([1, 512], F32, tag="br" + name)
        (eng or nc.sync).dma_start(brow[:], io[name][:])
        bb = spsum.tile([BL, 512], F32, tag="sp")
        nc.tensor.matmul(bb[:], ones[:], brow[:], start=True, stop=True)
        bsb = pool.tile([BL, 512], F32, tag="bs" + name)
        nc.vector.tensor_copy(bsb[:], bb[:])
        return bsb

    bk2 = bias_bcast("brk2_flat")
    bv2 = bias_bcast("brv2_flat", eng=nc.scalar)

    def layer_nat(xT_chunks, w_name, n_out, eng=None):
        nk = len(xT_chunks)
        w = wpool.tile([128, nk, n_out], BF16, tag="Wstgb")
        (eng or nc.sync).dma_start(w[:], io[w_name][:])
        ps = spsum.tile([BL, n_out], F32, tag="sp")
        for k in range(nk):
            nc.tensor.matmul(ps[:], xT_chunks[k], w[:, k, :],
                             start=(k == 0), stop=(k == nk - 1),
                             skip_group_check=True)
        return ps

    hkT = layer_Tio(AT, "Wrk1", brk1, HID, "hk")
    ok_ps = layer_nat(hkT, "Wrk2", RIMQ)
    hvT = layer_Tio(AT, "Wrv1", brv1, HID, "hv", eng=nc.scalar)
    ov_ps = layer_nat(hvT, "Wrv2", VD, eng=nc.scalar)

    for name, ps_, bias_sb in (("out_key", ok_ps, bk2), ("out_val", ov_ps, bv2)):
        onat = pool.tile([BL, 512], F32, tag="o" + name)
        nc.vector.tensor_tensor(out=onat[:], in0=ps_[:], in1=bias_sb[:],
                                op=OP.add)
        nc.sync.dma_start(io[name][:], onat[:])


def _build(cstar):
    seqc = [cstar[s] for s in SEQ]
    NCH = int(sum(seqc))
    W = NCH * 128
    NW = (NCH + 3) // 4
    nc = bacc.Bacc("TRN2", target_bir_lowering=False, debug=False,
                   num_devices=NCORES)
    io = {}

    def din(name, shape, dt=BF16):
        io[name] = nc.dram_tensor(name, shape, dt, kind="ExternalInput").ap()

    din("keysT", [128, 2, 2, W], FP8)
    din("vals", [128, NCH, VD])
    din("A8", [128, 5168], FP8)
    din("CF", [128, NCH + 54], F32)
    din("Wq", [128, 2, 2, H * KD], FP8)
    din("Wagg", [128, 32, VD])
    din("Wrk1", [128, KC, HID])
    din("Wrk2", [128, KC, RIMQ])
    din("brk2_flat", [1, 512], F32)
    din("Wrv1", [128, KC, HID])
    din("Wrv2", [128, KC, VD])
    din("brv2_flat", [1, 512], F32)
    io["out_key"] = nc.dram_tensor("out_key", [BL, RIMQ], F32,
                                   kind="ExternalOutput").ap()
    io["out_val"] = nc.dram_tensor("out_val", [BL, VD], F32,
                                   kind="ExternalOutput").ap()

    with tile.TileContext(nc) as tc, ExitStack() as ctx:
        _emit(nc, tc, ctx, io, cstar)
    nc.compile()
    return nc


def _rsb(bias, nch, scale=1.0):
    return np.ascontiguousarray(
        np.asarray(bias, np.float32).reshape(nch, 128).T * scale)


def _wchunk(w, dt=NBF16, scale=1.0):
    w = np.asarray(w, np.float32) * scale
    f, c = w.shape
    return np.ascontiguousarray(
        w.reshape(f // 128, 128, c).transpose(1, 0, 2)).astype(dt)


def _actT(x, dt):
    x = np.asarray(x, np.float32)
    bl, f = x.shape
    return np.ascontiguousarray(
        x.T.reshape(f // 128, 128, bl).transpose(1, 0, 2)).astype(dt)


def _plan(step):
    cb = np.clip((np.asarray(step, np.int64) + 127) // 128, 1, 8)
    order = np.argsort(-cb, kind="stable")
    cstar = tuple(int(cb[order[8 * s]]) for s in range(BL))
    return order, cstar


def _shard(inputs):
    f = lambda x: np.asarray(x, np.float32)
    keys, vals, rpe = f(inputs["keys"]), f(inputs["vals"]), f(inputs["rpe_mod"])
    step = np.asarray(inputs["step"]).astype(np.int64)
    state, lat = f(inputs["state"]), f(inputs["task_inference_latent"])

    order, cstar = _plan(step)
    seqc = [cstar[s] for s in SEQ]
    offs = np.concatenate([[0], np.cumsum(seqc)])
    NCH = int(offs[-1])
    NW = (NCH + 3) // 4

    A8w = np.concatenate([
        _wchunk(inputs["W_state"], NFP8, WSCALE).reshape(128, -1),
        _wchunk(inputs["Wcq1"], NFP8, WSCALE).reshape(128, -1),
        _wchunk(inputs["Wcq2"], NFP8, WSCALE).reshape(128, -1)], axis=1)
    cf_tail = np.concatenate([
        _rsb(inputs["b_state"], 2, WSCALE),
        _rsb(inputs["bcq1"], 4, WSCALE),
        _rsb(inputs["bcq2"], 4, WSCALE),
        _rsb(inputs["bq"], 32, WSCALE * QCS),
        _rsb(inputs["bagg"], 4),
        _rsb(inputs["brk1"], 4),
        _rsb(inputs["brv1"], 4)], axis=1).astype(np.float32)
    shared = {
        "Wq": _wchunk(inputs["Wq"], NFP8, WSCALE).reshape(128, 2, 2, H * KD),
        "Wagg": _wchunk(inputs["Wagg"]),
        "Wrk1": _wchunk(inputs["Wrk1"]),
        "Wrk2": _wchunk(inputs["Wrk2"]),
        "brk2_flat": np.ascontiguousarray(f(inputs["brk2"])[None, :]),
        "Wrv1": _wchunk(inputs["Wrv1"]),
        "Wrv2": _wchunk(inputs["Wrv2"]),
        "brv2_flat": np.ascontiguousarray(f(inputs["brv2"])[None, :]),
    }
    kfold = keys * rpe * (KSCALE * RSQK)            # [L, 64, K]
    in_maps = []
    for m in range(NCORES):
        envs = [int(order[8 * s + m]) for s in range(BL)]
        kp = np.zeros((128, 2, 2, NCH * 128), NFP8)
        vp = np.zeros((128, NCH, VD), NBF16)
        rowbias = np.zeros((128, NCH), np.float32)
        for p, s in enumerate(SEQ):
            e = envs[s]
            nl = cstar[s] * 128
            c0, c1 = int(offs[p]), int(offs[p + 1])
            kb = kfold[:nl, e, :].T.reshape(2, 2, 128, nl).transpose(
                2, 0, 1, 3)
            kp[:, :, :, c0 * 128:c1 * 128] = kb.astype(NFP8)
            vb = vals[:nl, e, :].reshape(cstar[s], 128, VD).transpose(1, 0, 2)
            vp[:, c0:c1, :] = vb.astype(NBF16)
            labs = (np.arange(128)[:, None]
                    + 128 * np.arange(c1 - c0)[None, :])
            rowbias[:, c0:c1] = np.where(labs < int(step[e]), 0.0, -1e30)
        a8 = np.concatenate([
            _actT(state[envs], NFP8).reshape(128, -1),
            _actT(lat[envs], NFP8).reshape(128, -1),
            A8w], axis=1)
        cf = np.concatenate([rowbias, cf_tail], axis=1).astype(np.float32)
        in_maps.append({
            "keysT": kp, "vals": vp, "A8": np.ascontiguousarray(a8),
            "CF": np.ascontiguousarray(cf),
            **shared,
        })
    return in_maps, order


def kernel(**inputs):
    order, cstar = _plan(inputs["step"])
    nc = _CACHE.get(cstar)
    if nc is None:
        nc = _CACHE[cstar] = _build(cstar)
    in_maps, order = _shard(inputs)
    res = run_bass_kernel_spmd(nc, in_maps, list(range(NCORES)),
                               **_CACHE.get("run_kwargs", {}))
    _CACHE["last_result"] = res
    ok = np.empty((B, RIMQ), np.float32)
    ov = np.empty((B, VD), np.float32)
    for m in range(NCORES):
        for s in range(BL):
            e = int(order[8 * s + m])
            ok[e] = res.results[m]["out_key"][s]
            ov[e] = res.results[m]["out_val"][s]
    return ok[:, None, :], ov[:, None, :]


# revision 40
# speedup vs baseline: 1.1000x; 1.0625x over previous
"""DND retrieval (episodic memory read) kernel for 8 Trainium2 NeuronCores.

Data-parallel over batch B=64 -> 8 envs per core, with step-aware
packing: only ceil(step/128) l-chunks per env are ever touched (the
rest are masked to zero by the softmax validity mask), so the host
packs exactly those chunks, assigns envs to cores by sorted rank so
every core shares one compiled chunk pattern C*, and the kernel skips
the dead ~45% of keys/vals DMA and PE work.

Precision: keys (with rpe * 64/sqrt(K) folded in) and the q-side MLP
stream as fp8e4m3 (weights x32, qc x32, q x16 host/chip scales); the
scores and Wq matmuls run in fp8 DoubleRow mode (2 contraction rows
per partition, 2x PE rate). vals and output-side weights stay bf16
(fp8 there pushes error past budget).

Scores are processed in 512-column windows of the packed image through
a 2-bank PSUM ring: scores -> exp(S/1024) -> multiply by a precomputed
validity mask -> unnormalized probs transpose straight into the value
matmul; softmax 1/Z is applied to the [64, 512] result instead
(linearity), so nothing waits on the global sum. Scores are tiny
(|s| < 0.3), so no max pass is needed.
"""
from contextlib import ExitStack

import numpy as np
import ml_dtypes

import concourse.bass as bass
import concourse.tile as tile
from concourse import bacc, mybir
from concourse.bass_utils import run_bass_kernel_spmd
from concourse.masks import make_identity

F32 = mybir.dt.float32
BF16 = mybir.dt.bfloat16
FP8 = mybir.dt.float8e4
AF = mybir.ActivationFunctionType
OP = mybir.AluOpType
DR = mybir.MatmulPerfMode.DoubleRow

L = 1024
B = 64        # rows of the batched softmax image: (slot, head)
BL = 8        # envs (slots) per core
KD = 512
VD = 512
H = 8
MEMB = 256
SDIM = 512
HID = 512
RIMQ = 512
LAT = KD - MEMB
NCORES = 8
KC = KD // 128
RSQK = 1.0 / np.sqrt(np.float32(KD))
KSCALE = 64.0          # folded into keys on host
WSCALE = 32.0          # fp8 weight scale
QCS = 32.0             # qc activation fp8 scale
QS = 16.0              # q fp8 scale inside Qpad
NBF16 = np.dtype(ml_dtypes.bfloat16)
NFP8 = np.dtype(ml_dtypes.float8_e4m3)
SEQ = [0, 7, 1, 6, 2, 5, 3, 4]   # packed slot order

_CACHE: dict = {}


def _emit(nc: bass.Bass, tc: tile.TileContext, ctx: ExitStack, io: dict,
          cstar: tuple):
    # ---- packed geometry (compile-time) ----
    seqc = [cstar[s] for s in SEQ]
    offs = np.concatenate([[0], np.cumsum(seqc)])
    NCH = int(offs[-1])
    W = NCH * 128
    owner = []                       # chunk idx -> slot
    for p, s in enumerate(SEQ):
        owner += [s] * seqc[p]
    NW = (NCH + 3) // 4              # 512-col score windows
    NS = (NW + 1) // 2               # keys DMA slabs (2 windows each)

    pool = ctx.enter_context(tc.tile_pool(name="main", bufs=1))
    kpool = ctx.enter_context(tc.tile_pool(name="keys", bufs=2 * NS))
    wpool = ctx.enter_context(tc.tile_pool(name="wstream", bufs=2))
    psum = ctx.enter_context(tc.tile_pool(name="ps", bufs=3, space="PSUM"))
    spsum = ctx.enter_context(tc.tile_pool(name="ps2", bufs=2, space="PSUM"))
    rpsum = ctx.enter_context(tc.tile_pool(name="ps3", bufs=1, space="PSUM"))

    identb = pool.tile([128, 128], BF16)
    make_identity(nc, identb[:])

    def bias_tile(name, nch, eng=None):
        t = pool.tile([128, nch], F32, tag="b" + name)
        (eng or nc.sync).dma_start(t[:], io[name][:])
        return t

    # One fp8 blob (5 KB lines) carries all phase-A operands: the former
    # per-tensor DMAs had 8-140 B partition lines whose descriptor overhead
    # stalled the sync queue ~25 us before W_state even started.
    A8 = pool.tile([128, 5168], FP8)
    nc.sync.dma_start(A8[:], io["A8"][:])
    CF = pool.tile([128, NCH + 54], F32)
    nc.sync.dma_start(CF[:], io["CF"][:])
    rowbias = CF[:, 0:NCH]
    bst = CF[:, NCH:NCH + 2]
    bcq1 = CF[:, NCH + 2:NCH + 6]
    bcq2 = CF[:, NCH + 6:NCH + 10]
    bq = CF[:, NCH + 10:NCH + 42]

    # ---------------- Phase A: q-side MLP (fp8, DoubleRow Wq) -------------
    stateT_n = A8[:, 0:32].rearrange("p (k b) -> p k b", k=4)
    latT_n = A8[:, 32:48].rearrange("p (k b) -> p k b", k=2)
    w_state = A8[:, 48:1072].rearrange("p (k c) -> p k c", k=4)
    w_cq1 = A8[:, 1072:3120].rearrange("p (k c) -> p k c", k=4)
    w_cq2 = A8[:, 3120:5168].rearrange("p (k c) -> p k c", k=4)

    stateT = [stateT_n[:, c, :] for c in range(SDIM // 128)]
    latT = [latT_n[:, c, :] for c in range(LAT // 128)]

    def layer_T(xT_chunks, w, b_tile, n_out, tag, scale=None):
        nk = len(xT_chunks)
        outs = []
        for j in range(n_out // 128):
            ps = psum.tile([128, BL], F32, tag="sm")
            for k in range(nk):
                nc.tensor.matmul(ps[:], w[:, k, j * 128:(j + 1) * 128],
                                 xT_chunks[k], start=(k == 0),
                                 stop=(k == nk - 1), skip_group_check=True)
            t = pool.tile([128, BL], BF16, tag=f"{tag}{j}")
            if scale is None:
                nc.vector.tensor_scalar(out=t[:], in0=ps[:],
                                        scalar1=b_tile[:, j:j + 1],
                                        scalar2=None, op0=OP.add)
            else:
                nc.vector.tensor_scalar(out=t[:], in0=ps[:],
                                        scalar1=b_tile[:, j:j + 1],
                                        scalar2=scale, op0=OP.add,
                                        op1=OP.mult)
            outs.append(t[:])
        return outs

    def layer_Tio(xT_chunks, w_name, b_tile, n_out, tag, eng=None):
        nk = len(xT_chunks)
        w = wpool.tile([128, nk, n_out], BF16, tag="Wstgb")
        (eng or nc.sync).dma_start(w[:], io[w_name][:])
        return layer_T(xT_chunks, w[:], b_tile, n_out, tag)

    RW = 1.0 / WSCALE
    xT = layer_T(stateT, w_state, bst, MEMB, "xT", scale=RW) + latT
    h1T = layer_T(xT, w_cq1, bcq1, HID, "h1", scale=RW)
    # qc layer -> single fp8 tile (x QCS), consumed as DoubleRow lhsT.
    # Padded to QCW columns: dual-fp8 LDWEIGHTS rejects 8-wide loads.
    QCW = 32
    qcT = pool.tile([128, KC, QCW], FP8)
    nc.gpsimd.memset(qcT[:], 0.0)
    for j in range(KC):
        ps = psum.tile([128, BL], F32, tag="sm")
        for k in range(KC):
            nc.tensor.matmul(ps[:], w_cq2[:, k, j * 128:(j + 1) * 128],
                             h1T[k], start=(k == 0), stop=(k == KC - 1),
                             skip_group_check=True)
        nc.vector.tensor_scalar(out=qcT[:, j, 0:BL], in0=ps[:],
                                scalar1=bcq2[:, j:j + 1], scalar2=QCS / 32.0,
                                op0=OP.add, op1=OP.mult)

    # Wq in DoubleRow fp8: out [8, 512] per (jg, kcp), then transpose and
    # scatter into Qpad (fp8, xQS) diagonal windows.
    Qpad = pool.tile([128, 2, 2, BL, 72], FP8)
    nc.gpsimd.memset(Qpad[:], 0.0)
    wq = pool.tile([128, 2, 2, H * KD], FP8)
    for kcp in range(2):
        (nc.sync if kcp == 0 else nc.scalar).dma_start(
            wq[:, kcp, :, :], io["Wq"][:, kcp, :, :])
    QSC = QS / (32.0 * QCS)
    for jg in range(8):
        ps = spsum.tile([QCW, 512], F32, tag="sp")
        for kcp in range(2):
            nc.tensor.matmul(ps[:], qcT[:, 2 * kcp:2 * kcp + 2, :],
                             wq[:, kcp, :, jg * 512:(jg + 1) * 512],
                             start=(kcp == 0), stop=(kcp == 1),
                             perf_mode=DR, skip_group_check=True)
        qsb = pool.tile([BL, 512], BF16, tag="qsb")
        nc.scalar.copy(qsb[:], ps[0:BL, :])
        for jj in range(4):
            j = jg * 4 + jj
            h, kc = j // KC, j % KC
            tp = psum.tile([128, BL], BF16, tag="sm")
            nc.tensor.transpose(tp[:], qsb[:, jj * 128:(jj + 1) * 128],
                                identb[0:BL, 0:BL])
            nc.vector.tensor_scalar(
                out=Qpad[:, kc // 2, kc % 2, :, h], in0=tp[:],
                scalar1=bq[:, j:j + 1], scalar2=QSC, op0=OP.add, op1=OP.mult)

    # ------- keys first (all slabs), then vals; queues never block keys ---
    slabs = []
    vres = pool.tile([128, NCH, VD], BF16)
    for si in range(NS):
        c0, c1 = 8 * si, min(8 * si + 8, NCH)
        kts = []
        for kcp in range(2):
            kt = kpool.tile([128, 2, 1024], FP8, tag="kt")
            (nc.sync if kcp == 0 else nc.scalar).dma_start(
                kt[:, :, 0:(c1 - c0) * 128],
                io["keysT"][:, kcp, :, c0 * 128:c1 * 128])
            kts.append(kt)
        slabs.append(kts)
    vengs = [nc.gpsimd, nc.gpsimd, nc.scalar, nc.sync]
    for si in range(NS):
        c0, c1 = 8 * si, min(8 * si + 8, NCH)
        vengs[si % 4].dma_start(vres[:, c0:c1, :], io["vals"][:, c0:c1, :])
    wagg = pool.tile([128, 32, VD], BF16)
    waeng = [nc.gpsimd, nc.scalar, nc.gpsimd, nc.sync]
    for gi in range(4):
        waeng[gi].dma_start(wagg[:, gi * 8:(gi + 1) * 8, :],
                            io["Wagg"][:, gi * 8:(gi + 1) * 8, :])

    # ------- per-chunk: scoresT -> exp(+bias) -> value matmul + Z ----------
    # scoresT [128(l), 8(h)] per chunk; exp writes masked unnormalized probs
    # straight into the transposed EVT image the value matmul consumes.
    EVT = pool.tile([128, NCH, B], BF16)
    nc.gpsimd.memset(EVT[:], 0.0)
    onesb = pool.tile([128, 1], BF16)
    nc.gpsimd.memset(onesb[:], 1.0)
    rps = rpsum.tile([B, VD], F32, tag="rp")
    zps = rpsum.tile([B, 1], F32, tag="z")

    def chunkwork(i):
        nc.tensor.matmul(rps[:], EVT[:, i, :], vres[:, i, :],
                         start=(i == 0), stop=(i == NCH - 1),
                         skip_group_check=True)
        nc.tensor.matmul(zps[:], EVT[:, i, :], onesb[:],
                         start=(i == 0), stop=(i == NCH - 1),
                         skip_group_check=True)

    for i in range(NCH):
        s = owner[i]
        si, sc0 = i // 8, (i % 8) * 128
        kts = slabs[si]
        sgt = psum.tile([128, H], F32, tag="sm")
        for kcp in range(2):
            nc.tensor.matmul(sgt[:], kts[kcp][:, :, sc0:sc0 + 128],
                             Qpad[:, kcp, :, s, 0:H],
                             start=(kcp == 0), stop=(kcp == 1),
                             perf_mode=DR, skip_group_check=True)
        nc.scalar.activation(EVT[:, i, s * H:(s + 1) * H], sgt[:], AF.Exp,
                             bias=rowbias[:, i:i + 1],
                             scale=1.0 / (KSCALE * QS))
        if i > 1:
            chunkwork(i - 2)
    for i in range(max(NCH - 2, 0), NCH):
        chunkwork(i)

    # R = 1/Z folded into the result readout
    R = pool.tile([B, 1], F32)
    nc.vector.reciprocal(R[:], zps[:])
    rsb = pool.tile([B, VD], BF16, tag="rs")
    nc.vector.tensor_scalar(out=rsb[:], in0=rps[:], scalar1=R[:, 0:1],
                            scalar2=None, op0=OP.mult)
    RT = pool.tile([128, VD // 128, B], BF16)
    for vc in range(VD // 128):
        tr = psum.tile([128, B], BF16, tag="sm")
        nc.tensor.transpose(tr[:], rsb[:, vc * 128:(vc + 1) * 128],
                            identb[0:B, 0:B])
        nc.vector.tensor_copy(RT[:, vc, :], tr[:])

    # ---------------- Phase E: output MLP chain (bf16) ---------------------
    bagg = CF[:, NCH + 42:NCH + 46]
    brk1 = CF[:, NCH + 46:NCH + 50]
    brv1 = CF[:, NCH + 50:NCH + 54]

    aggp = spsum.tile([BL, VD], F32, tag="sp")
    for c in range(32):
        h, vc = c // 4, c % 4
        nc.tensor.matmul(aggp[:], RT[:, vc, h:B:H], wagg[:, c, :],
                         start=(c == 0), stop=(c == 31),
                         skip_group_check=True)
    aggsb = pool.tile([BL, VD], BF16, tag="aggsb")
    nc.scalar.copy(aggsb[:], aggp[:])
    AT = []
    for j in range(VD // 128):
        tp = psum.tile([128, BL], BF16, tag="sm")
        nc.tensor.transpose(tp[:], aggsb[:, j * 128:(j + 1) * 128],
                            identb[0:BL, 0:BL])
        t = pool.tile([128, BL], BF16, tag=f"AT{j}")
        nc.vector.tensor_scalar(out=t[:], in0=tp[:],
                                scalar1=bagg[:, j:j + 1],
                                scalar2=None, op0=OP.add)
        AT.append(t[:])

    ones = pool.tile([1, BL], F32)
    nc.gpsimd.memset(ones[:], 1.0)

    def bias_bcast(name, eng=None):
        brow = pool.tile([1, 512], F32, tag="br" + name)
        (eng or nc.sync).dma_start(brow[:], io[name][:])
        bb = spsum.tile([BL, 512], F32, tag="sp")
        nc.tensor.matmul(bb[:], ones[:], brow[:], start=True, stop=True)
        bsb = pool.tile([BL, 512], F32, tag="bs" + name)
        nc.vector.tensor_copy(bsb[:], bb[:])
        return bsb

    bk2 = bias_bcast("brk2_flat")
    bv2 = bias_bcast("brv2_flat", eng=nc.scalar)

    def layer_nat(xT_chunks, w_name, n_out, eng=None):
        nk = len(xT_chunks)
        w = wpool.tile([128, nk, n_out], BF16, tag="Wstgb")
        (eng or nc.sync).dma_start(w[:], io[w_name][:])
        ps = spsum.tile([BL, n_out], F32, tag="sp")
        for k in range(nk):
            nc.tensor.matmul(ps[:], xT_chunks[k], w[:, k, :],
                             start=(k == 0), stop=(k == nk - 1),
                             skip_group_check=True)
        return ps

    hkT = layer_Tio(AT, "Wrk1", brk1, HID, "hk")
    ok_ps = layer_nat(hkT, "Wrk2", RIMQ)
    hvT = layer_Tio(AT, "Wrv1", brv1, HID, "hv", eng=nc.scalar)
    ov_ps = layer_nat(hvT, "Wrv2", VD, eng=nc.scalar)

    for name, ps_, bias_sb in (("out_key", ok_ps, bk2), ("out_val", ov_ps, bv2)):
        onat = pool.tile([BL, 512], F32, tag="o" + name)
        nc.vector.tensor_tensor(out=onat[:], in0=ps_[:], in1=bias_sb[:],
                                op=OP.add)
        nc.sync.dma_start(io[name][:], onat[:])


def _build(cstar):
    seqc = [cstar[s] for s in SEQ]
    NCH = int(sum(seqc))
    W = NCH * 128
    NW = (NCH + 3) // 4
    nc = bacc.Bacc("TRN2", target_bir_lowering=False, debug=False,
                   num_devices=NCORES)
    io = {}

    def din(name, shape, dt=BF16):
        io[name] = nc.dram_tensor(name, shape, dt, kind="ExternalInput").ap()

    din("keysT", [128, 2, 2, W], FP8)
    din("vals", [128, NCH, VD])
    din("A8", [128, 5168], FP8)
    din("CF", [128, NCH + 54], F32)
    din("Wq", [128, 2, 2, H * KD], FP8)
    din("Wagg", [128, 32, VD])
    din("Wrk1", [128, KC, HID])
    din("Wrk2", [128, KC, RIMQ])
    din("brk2_flat", [1, 512], F32)
    din("Wrv1", [128, KC, HID])
    din("Wrv2", [128, KC, VD])
    din("brv2_flat", [1, 512], F32)
    io["out_key"] = nc.dram_tensor("out_key", [BL, RIMQ], F32,
                                   kind="ExternalOutput").ap()
    io["out_val"] = nc.dram_tensor("out_val", [BL, VD], F32,
                                   kind="ExternalOutput").ap()

    with tile.TileContext(nc) as tc, ExitStack() as ctx:
        _emit(nc, tc, ctx, io, cstar)
    nc.compile()
    return nc


def _rsb(bias, nch, scale=1.0):
    return np.ascontiguousarray(
        np.asarray(bias, np.float32).reshape(nch, 128).T * scale)


def _wchunk(w, dt=NBF16, scale=1.0):
    w = np.asarray(w, np.float32) * scale
    f, c = w.shape
    return np.ascontiguousarray(
        w.reshape(f // 128, 128, c).transpose(1, 0, 2)).astype(dt)


def _actT(x, dt):
    x = np.asarray(x, np.float32)
    bl, f = x.shape
    return np.ascontiguousarray(
        x.T.reshape(f // 128, 128, bl).transpose(1, 0, 2)).astype(dt)


def _plan(step):
    cb = np.clip((np.asarray(step, np.int64) + 127) // 128, 1, 8)
    order = np.argsort(-cb, kind="stable")
    cstar = tuple(int(cb[order[8 * s]]) for s in range(BL))
    return order, cstar


def _shard(inputs):
    f = lambda x: np.asarray(x, np.float32)
    keys, vals, rpe = f(inputs["keys"]), f(inputs["vals"]), f(inputs["rpe_mod"])
    step = np.asarray(inputs["step"]).astype(np.int64)
    state, lat = f(inputs["state"]), f(inputs["task_inference_latent"])

    order, cstar = _plan(step)
    seqc = [cstar[s] for s in SEQ]
    offs = np.concatenate([[0], np.cumsum(seqc)])
    NCH = int(offs[-1])
    NW = (NCH + 3) // 4

    A8w = np.concatenate([
        _wchunk(inputs["W_state"], NFP8, WSCALE).reshape(128, -1),
        _wchunk(inputs["Wcq1"], NFP8, WSCALE).reshape(128, -1),
        _wchunk(inputs["Wcq2"], NFP8, WSCALE).reshape(128, -1)], axis=1)
    cf_tail = np.concatenate([
        _rsb(inputs["b_state"], 2, WSCALE),
        _rsb(inputs["bcq1"], 4, WSCALE),
        _rsb(inputs["bcq2"], 4, WSCALE),
        _rsb(inputs["bq"], 32, WSCALE * QCS),
        _rsb(inputs["bagg"], 4),
        _rsb(inputs["brk1"], 4),
        _rsb(inputs["brv1"], 4)], axis=1).astype(np.float32)
    shared = {
        "Wq": _wchunk(inputs["Wq"], NFP8, WSCALE).reshape(128, 2, 2, H * KD),
        "Wagg": _wchunk(inputs["Wagg"]),
        "Wrk1": _wchunk(inputs["Wrk1"]),
        "Wrk2": _wchunk(inputs["Wrk2"]),
        "brk2_flat": np.ascontiguousarray(f(inputs["brk2"])[None, :]),
        "Wrv1": _wchunk(inputs["Wrv1"]),
        "Wrv2": _wchunk(inputs["Wrv2"]),
        "brv2_flat": np.ascontiguousarray(f(inputs["brv2"])[None, :]),
    }
    kfold = keys * rpe * (KSCALE * RSQK)            # [L, 64, K]
    in_maps = []
    for m in range(NCORES):
        envs = [int(order[8 * s + m]) for s in range(BL)]
        kp = np.zeros((128, 2, 2, NCH * 128), NFP8)
        vp = np.zeros((128, NCH, VD), NBF16)
        rowbias = np.zeros((128, NCH), np.float32)
        for p, s in enumerate(SEQ):
            e = envs[s]
            nl = cstar[s] * 128
            c0, c1 = int(offs[p]), int(offs[p + 1])
            kb = kfold[:nl, e, :].T.reshape(2, 2, 128, nl).transpose(
                2, 0, 1, 3)
            kp[:, :, :, c0 * 128:c1 * 128] = kb.astype(NFP8)
            vb = vals[:nl, e, :].reshape(cstar[s], 128, VD).transpose(1, 0, 2)
            vp[:, c0:c1, :] = vb.astype(NBF16)
            labs = (np.arange(128)[:, None]
                    + 128 * np.arange(c1 - c0)[None, :])
            rowbias[:, c0:c1] = np.where(labs < int(step[e]), 0.0, -1e30)
        a8 = np.concatenate([
            _actT(state[envs], NFP8).reshape(128, -1),
            _actT(lat[envs], NFP8).reshape(128, -1),
            A8w], axis=1)
        cf = np.concatenate([rowbias, cf_tail], axis=1).astype(np.float32)
        in_maps.append({
            "keysT": kp, "vals": vp, "A8": np.ascontiguousarray(a8),
            "CF": np.ascontiguousarray(cf),
            **shared,
        })
    return in_maps, order


def kernel(**inputs):
    order, cstar = _plan(inputs["step"])
    nc = _CACHE.get(cstar)
    if nc is None:
        nc = _CACHE[cstar] = _build(cstar)
    in_maps, order = _shard(inputs)
    res = run_bass_kernel_spmd(nc, in_maps, list(range(NCORES)),
                               **_CACHE.get("run_kwargs", {}))
    _CACHE["last_result"] = res
    ok = np.empty((B, RIMQ), np.float32)
    ov = np.empty((B, VD), np.float32)
    for m in range(NCORES):
        for s in range(BL):
            e = int(order[8 * s + m])
            ok[e] = res.results[m]["out_key"][s]
            ov[e] = res.results[m]["out_val"][s]
    return ok[:, None, :], ov[:, None, :]


# revision 42
# speedup vs baseline: 1.1294x; 1.0267x over previous
"""DND retrieval (episodic memory read) kernel for 8 Trainium2 NeuronCores.

Data-parallel over batch B=64 -> 8 envs per core, with step-aware
packing: only ceil(step/128) l-chunks per env are ever touched (the
rest are masked to zero by the softmax validity mask), so the host
packs exactly those chunks, assigns envs to cores by sorted rank so
every core shares one compiled chunk pattern C*, and the kernel skips
the dead ~45% of keys/vals DMA and PE work.

Precision: keys (with rpe * 64/sqrt(K) folded in) and the q-side MLP
stream as fp8e4m3 (weights x32, qc x32, q x16 host/chip scales); the
scores and Wq matmuls run in fp8 DoubleRow mode (2 contraction rows
per partition, 2x PE rate). vals and output-side weights stay bf16
(fp8 there pushes error past budget).

Scores are processed in 512-column windows of the packed image through
a 2-bank PSUM ring: scores -> exp(S/1024) -> multiply by a precomputed
validity mask -> unnormalized probs transpose straight into the value
matmul; softmax 1/Z is applied to the [64, 512] result instead
(linearity), so nothing waits on the global sum. Scores are tiny
(|s| < 0.3), so no max pass is needed.
"""
from contextlib import ExitStack

import numpy as np
import ml_dtypes

import concourse.bass as bass
import concourse.tile as tile
from concourse import bacc, mybir
from concourse.bass_utils import run_bass_kernel_spmd
from concourse.masks import make_identity

F32 = mybir.dt.float32
BF16 = mybir.dt.bfloat16
FP8 = mybir.dt.float8e4
AF = mybir.ActivationFunctionType
OP = mybir.AluOpType
DR = mybir.MatmulPerfMode.DoubleRow

L = 1024
B = 64        # rows of the batched softmax image: (slot, head)
BL = 8        # envs (slots) per core
KD = 512
VD = 512
H = 8
MEMB = 256
SDIM = 512
HID = 512
RIMQ = 512
LAT = KD - MEMB
NCORES = 8
KC = KD // 128
RSQK = 1.0 / np.sqrt(np.float32(KD))
KSCALE = 64.0          # folded into keys on host
WSCALE = 32.0          # fp8 weight scale
QCS = 32.0             # qc activation fp8 scale
QS = 16.0              # q fp8 scale inside Qpad
NBF16 = np.dtype(ml_dtypes.bfloat16)
NFP8 = np.dtype(ml_dtypes.float8_e4m3)
SEQ = [0, 7, 1, 6, 2, 5, 3, 4]   # packed slot order

_CACHE: dict = {}


def _emit(nc: bass.Bass, tc: tile.TileContext, ctx: ExitStack, io: dict,
          cstar: tuple):
    # ---- packed geometry (compile-time) ----
    seqc = [cstar[s] for s in SEQ]
    offs = np.concatenate([[0], np.cumsum(seqc)])
    NCH = int(offs[-1])
    W = NCH * 128
    owner = []                       # chunk idx -> slot
    for p, s in enumerate(SEQ):
        owner += [s] * seqc[p]
    NW = (NCH + 3) // 4              # 512-col score windows
    NS = (NW + 1) // 2               # keys DMA slabs (2 windows each)

    pool = ctx.enter_context(tc.tile_pool(name="main", bufs=1))
    kpool = ctx.enter_context(tc.tile_pool(name="keys", bufs=2 * NS))
    wpool = ctx.enter_context(tc.tile_pool(name="wstream", bufs=2))
    psum = ctx.enter_context(tc.tile_pool(name="ps", bufs=3, space="PSUM"))
    spsum = ctx.enter_context(tc.tile_pool(name="ps2", bufs=2, space="PSUM"))
    rpsum = ctx.enter_context(tc.tile_pool(name="ps3", bufs=1, space="PSUM"))

    identb = pool.tile([128, 128], BF16)
    make_identity(nc, identb[:])

    def bias_tile(name, nch, eng=None):
        t = pool.tile([128, nch], F32, tag="b" + name)
        (eng or nc.sync).dma_start(t[:], io[name][:])
        return t

    # One fp8 blob (5 KB lines) carries all phase-A operands: the former
    # per-tensor DMAs had 8-140 B partition lines whose descriptor overhead
    # stalled the sync queue ~25 us before W_state even started.
    A8 = pool.tile([128, 5168], FP8)
    nc.sync.dma_start(A8[:], io["A8"][:])
    CF = pool.tile([128, NCH + 54], F32)
    nc.sync.dma_start(CF[:], io["CF"][:])
    rowbias = CF[:, 0:NCH]
    bst = CF[:, NCH:NCH + 2]
    bcq1 = CF[:, NCH + 2:NCH + 6]
    bcq2 = CF[:, NCH + 6:NCH + 10]
    bq = CF[:, NCH + 10:NCH + 42]

    # ---------------- Phase A: q-side MLP (fp8, DoubleRow Wq) -------------
    stateT_n = A8[:, 0:32].rearrange("p (k b) -> p k b", k=4)
    latT_n = A8[:, 32:48].rearrange("p (k b) -> p k b", k=2)
    w_state = A8[:, 48:1072].rearrange("p (k c) -> p k c", k=4)
    w_cq1 = A8[:, 1072:3120].rearrange("p (k c) -> p k c", k=4)
    w_cq2 = A8[:, 3120:5168].rearrange("p (k c) -> p k c", k=4)

    stateT = [stateT_n[:, c, :] for c in range(SDIM // 128)]
    latT = [latT_n[:, c, :] for c in range(LAT // 128)]

    def layer_T(xT_chunks, w, b_tile, n_out, tag, scale=None):
        nk = len(xT_chunks)
        outs = []
        for j in range(n_out // 128):
            ps = psum.tile([128, BL], F32, tag="sm")
            for k in range(nk):
                nc.tensor.matmul(ps[:], w[:, k, j * 128:(j + 1) * 128],
                                 xT_chunks[k], start=(k == 0),
                                 stop=(k == nk - 1), skip_group_check=True)
            t = pool.tile([128, BL], BF16, tag=f"{tag}{j}")
            if scale is None:
                nc.vector.tensor_scalar(out=t[:], in0=ps[:],
                                        scalar1=b_tile[:, j:j + 1],
                                        scalar2=None, op0=OP.add)
            else:
                nc.vector.tensor_scalar(out=t[:], in0=ps[:],
                                        scalar1=b_tile[:, j:j + 1],
                                        scalar2=scale, op0=OP.add,
                                        op1=OP.mult)
            outs.append(t[:])
        return outs

    def layer_Tio(xT_chunks, w_name, b_tile, n_out, tag, eng=None):
        nk = len(xT_chunks)
        w = wpool.tile([128, nk, n_out], BF16, tag="Wstgb")
        (eng or nc.sync).dma_start(w[:], io[w_name][:])
        return layer_T(xT_chunks, w[:], b_tile, n_out, tag)

    RW = 1.0 / WSCALE
    xT = layer_T(stateT, w_state, bst, MEMB, "xT", scale=RW) + latT
    h1T = layer_T(xT, w_cq1, bcq1, HID, "h1", scale=RW)
    # qc layer -> single fp8 tile (x QCS), consumed as DoubleRow lhsT.
    # Padded to QCW columns: dual-fp8 LDWEIGHTS rejects 8-wide loads.
    QCW = 32
    qcT = pool.tile([128, KC, QCW], FP8)
    nc.gpsimd.memset(qcT[:], 0.0)
    for j in range(KC):
        ps = psum.tile([128, BL], F32, tag="sm")
        for k in range(KC):
            nc.tensor.matmul(ps[:], w_cq2[:, k, j * 128:(j + 1) * 128],
                             h1T[k], start=(k == 0), stop=(k == KC - 1),
                             skip_group_check=True)
        nc.vector.tensor_scalar(out=qcT[:, j, 0:BL], in0=ps[:],
                                scalar1=bcq2[:, j:j + 1], scalar2=QCS / 32.0,
                                op0=OP.add, op1=OP.mult)

    # Wq in DoubleRow fp8: out [8, 512] per (jg, kcp), then transpose and
    # scatter into Qpad (fp8, xQS) diagonal windows.
    Qpad = pool.tile([128, 2, 2, BL, 72], FP8)
    nc.gpsimd.memset(Qpad[:], 0.0)
    wq = pool.tile([128, 2, 2, H * KD], FP8)
    for kcp in range(2):
        (nc.sync if kcp == 0 else nc.scalar).dma_start(
            wq[:, kcp, :, :], io["Wq"][:, kcp, :, :])
    QSC = QS / (32.0 * QCS)
    for jg in range(8):
        ps = spsum.tile([QCW, 512], F32, tag="sp")
        for kcp in range(2):
            nc.tensor.matmul(ps[:], qcT[:, 2 * kcp:2 * kcp + 2, :],
                             wq[:, kcp, :, jg * 512:(jg + 1) * 512],
                             start=(kcp == 0), stop=(kcp == 1),
                             perf_mode=DR, skip_group_check=True)
        qsb = pool.tile([BL, 512], BF16, tag="qsb")
        nc.scalar.copy(qsb[:], ps[0:BL, :])
        for jj in range(4):
            j = jg * 4 + jj
            h, kc = j // KC, j % KC
            tp = psum.tile([128, BL], BF16, tag="sm")
            nc.tensor.transpose(tp[:], qsb[:, jj * 128:(jj + 1) * 128],
                                identb[0:BL, 0:BL])
            nc.vector.tensor_scalar(
                out=Qpad[:, kc // 2, kc % 2, :, h], in0=tp[:],
                scalar1=bq[:, j:j + 1], scalar2=QSC, op0=OP.add, op1=OP.mult)

    # ------- keys first (all slabs), then vals; queues never block keys ---
    slabs = []
    vres = pool.tile([128, NCH, VD], BF16)
    for si in range(NS):
        c0, c1 = 8 * si, min(8 * si + 8, NCH)
        kts = []
        for kcp in range(2):
            kt = kpool.tile([128, 2, 1024], FP8, tag="kt")
            (nc.sync if kcp == 0 else nc.scalar).dma_start(
                kt[:, :, 0:(c1 - c0) * 128],
                io["keysT"][:, kcp, :, c0 * 128:c1 * 128])
            kts.append(kt)
        slabs.append(kts)
    vengs = [nc.gpsimd, nc.scalar, nc.sync]
    for si in range(NS):
        c0, c1 = 8 * si, min(8 * si + 8, NCH)
        vengs[si % 3].dma_start(vres[:, c0:c1, :], io["vals"][:, c0:c1, :])
    wagg = pool.tile([128, 32, VD], BF16)
    waeng = [nc.gpsimd, nc.sync, nc.gpsimd, nc.sync]
    for gi in range(4):
        waeng[gi].dma_start(wagg[:, gi * 8:(gi + 1) * 8, :],
                            io["Wagg"][:, gi * 8:(gi + 1) * 8, :])

    # ------- per-chunk: scoresT -> exp(+bias) -> value matmul + Z ----------
    # scoresT [128(l), 8(h)] per chunk; exp writes masked unnormalized probs
    # straight into the transposed EVT image the value matmul consumes.
    EVT = pool.tile([128, NCH, B], BF16)
    nc.gpsimd.memset(EVT[:], 0.0)
    onesb = pool.tile([128, 1], BF16)
    nc.gpsimd.memset(onesb[:], 1.0)
    rps = rpsum.tile([B, VD], F32, tag="rp")
    zps = rpsum.tile([B, 1], F32, tag="z")

    def chunkwork(i):
        nc.tensor.matmul(rps[:], EVT[:, i, :], vres[:, i, :],
                         start=(i == 0), stop=(i == NCH - 1),
                         skip_group_check=True)
        nc.tensor.matmul(zps[:], EVT[:, i, :], onesb[:],
                         start=(i == 0), stop=(i == NCH - 1),
                         skip_group_check=True)

    for i in range(NCH):
        s = owner[i]
        si, sc0 = i // 8, (i % 8) * 128
        kts = slabs[si]
        sgt = psum.tile([128, H], F32, tag="sm")
        for kcp in range(2):
            nc.tensor.matmul(sgt[:], kts[kcp][:, :, sc0:sc0 + 128],
                             Qpad[:, kcp, :, s, 0:H],
                             start=(kcp == 0), stop=(kcp == 1),
                             perf_mode=DR, skip_group_check=True)
        nc.scalar.activation(EVT[:, i, s * H:(s + 1) * H], sgt[:], AF.Exp,
                             bias=rowbias[:, i:i + 1],
                             scale=1.0 / (KSCALE * QS))
        if i > 1:
            chunkwork(i - 2)
    for i in range(max(NCH - 2, 0), NCH):
        chunkwork(i)

    # R = 1/Z folded into the result readout
    R = pool.tile([B, 1], F32)
    nc.vector.reciprocal(R[:], zps[:])
    rsb = pool.tile([B, VD], BF16, tag="rs")
    nc.vector.tensor_scalar(out=rsb[:], in0=rps[:], scalar1=R[:, 0:1],
                            scalar2=None, op0=OP.mult)
    RT = pool.tile([128, VD // 128, B], BF16)
    for vc in range(VD // 128):
        tr = psum.tile([128, B], BF16, tag="sm")
        nc.tensor.transpose(tr[:], rsb[:, vc * 128:(vc + 1) * 128],
                            identb[0:B, 0:B])
        nc.vector.tensor_copy(RT[:, vc, :], tr[:])

    # ---------------- Phase E: output MLP chain (bf16) ---------------------
    bagg = CF[:, NCH + 42:NCH + 46]
    brk1 = CF[:, NCH + 46:NCH + 50]
    brv1 = CF[:, NCH + 50:NCH + 54]

    aggp = spsum.tile([BL, VD], F32, tag="sp")
    for c in range(32):
        h, vc = c // 4, c % 4
        nc.tensor.matmul(aggp[:], RT[:, vc, h:B:H], wagg[:, c, :],
                         start=(c == 0), stop=(c == 31),
                         skip_group_check=True)
    aggsb = pool.tile([BL, VD], BF16, tag="aggsb")
    nc.scalar.copy(aggsb[:], aggp[:])
    AT = []
    for j in range(VD // 128):
        tp = psum.tile([128, BL], BF16, tag="sm")
        nc.tensor.transpose(tp[:], aggsb[:, j * 128:(j + 1) * 128],
                            identb[0:BL, 0:BL])
        t = pool.tile([128, BL], BF16, tag=f"AT{j}")
        nc.vector.tensor_scalar(out=t[:], in0=tp[:],
                                scalar1=bagg[:, j:j + 1],
                                scalar2=None, op0=OP.add)
        AT.append(t[:])

    ones = pool.tile([1, BL], F32)
    nc.gpsimd.memset(ones[:], 1.0)

    def bias_bcast(name, eng=None):
        brow = pool.tile([1, 512], F32, tag="br" + name)
        (eng or nc.sync).dma_start(brow[:], io[name][:])
        bb = spsum.tile([BL, 512], F32, tag="sp")
        nc.tensor.matmul(bb[:], ones[:], brow[:], start=True, stop=True)
        bsb = pool.tile([BL, 512], F32, tag="bs" + name)
        nc.vector.tensor_copy(bsb[:], bb[:])
        return bsb

    bk2 = bias_bcast("brk2_flat")
    bv2 = bias_bcast("brv2_flat", eng=nc.scalar)

    def layer_nat(xT_chunks, w_name, n_out, eng=None):
        nk = len(xT_chunks)
        w = wpool.tile([128, nk, n_out], BF16, tag="Wstgb")
        (eng or nc.sync).dma_start(w[:], io[w_name][:])
        ps = spsum.tile([BL, n_out], F32, tag="sp")
        for k in range(nk):
            nc.tensor.matmul(ps[:], xT_chunks[k], w[:, k, :],
                             start=(k == 0), stop=(k == nk - 1),
                             skip_group_check=True)
        return ps

    hkT = layer_Tio(AT, "Wrk1", brk1, HID, "hk")
    ok_ps = layer_nat(hkT, "Wrk2", RIMQ)
    hvT = layer_Tio(AT, "Wrv1", brv1, HID, "hv", eng=nc.scalar)
    ov_ps = layer_nat(hvT, "Wrv2", VD, eng=nc.scalar)

    for name, ps_, bias_sb in (("out_key", ok_ps, bk2), ("out_val", ov_ps, bv2)):
        onat = pool.tile([BL, 512], F32, tag="o" + name)
        nc.vector.tensor_tensor(out=onat[:], in0=ps_[:], in1=bias_sb[:],
                                op=OP.add)
        nc.sync.dma_start(io[name][:], onat[:])


def _build(cstar):
    seqc = [cstar[s] for s in SEQ]
    NCH = int(sum(seqc))
    W = NCH * 128
    NW = (NCH + 3) // 4
    nc = bacc.Bacc("TRN2", target_bir_lowering=False, debug=False,
                   num_devices=NCORES)
    io = {}

    def din(name, shape, dt=BF16):
        io[name] = nc.dram_tensor(name, shape, dt, kind="ExternalInput").ap()

    din("keysT", [128, 2, 2, W], FP8)
    din("vals", [128, NCH, VD])
    din("A8", [128, 5168], FP8)
    din("CF", [128, NCH + 54], F32)
    din("Wq", [128, 2, 2, H * KD], FP8)
    din("Wagg", [128, 32, VD])
    din("Wrk1", [128, KC, HID])
    din("Wrk2", [128, KC, RIMQ])
    din("brk2_flat", [1, 512], F32)
    din("Wrv1", [128, KC, HID])
    din("Wrv2", [128, KC, VD])
    din("brv2_flat", [1, 512], F32)
    io["out_key"] = nc.dram_tensor("out_key", [BL, RIMQ], F32,
                                   kind="ExternalOutput").ap()
    io["out_val"] = nc.dram_tensor("out_val", [BL, VD], F32,
                                   kind="ExternalOutput").ap()

    with tile.TileContext(nc) as tc, ExitStack() as ctx:
        _emit(nc, tc, ctx, io, cstar)
    nc.compile()
    return nc


def _rsb(bias, nch, scale=1.0):
    return np.ascontiguousarray(
        np.asarray(bias, np.float32).reshape(nch, 128).T * scale)


def _wchunk(w, dt=NBF16, scale=1.0):
    w = np.asarray(w, np.float32) * scale
    f, c = w.shape
    return np.ascontiguousarray(
        w.reshape(f // 128, 128, c).transpose(1, 0, 2)).astype(dt)


def _actT(x, dt):
    x = np.asarray(x, np.float32)
    bl, f = x.shape
    return np.ascontiguousarray(
        x.T.reshape(f // 128, 128, bl).transpose(1, 0, 2)).astype(dt)


def _plan(step):
    cb = np.clip((np.asarray(step, np.int64) + 127) // 128, 1, 8)
    order = np.argsort(-cb, kind="stable")
    cstar = tuple(int(cb[order[8 * s]]) for s in range(BL))
    return order, cstar


def _shard(inputs):
    f = lambda x: np.asarray(x, np.float32)
    keys, vals, rpe = f(inputs["keys"]), f(inputs["vals"]), f(inputs["rpe_mod"])
    step = np.asarray(inputs["step"]).astype(np.int64)
    state, lat = f(inputs["state"]), f(inputs["task_inference_latent"])

    order, cstar = _plan(step)
    seqc = [cstar[s] for s in SEQ]
    offs = np.concatenate([[0], np.cumsum(seqc)])
    NCH = int(offs[-1])
    NW = (NCH + 3) // 4

    A8w = np.concatenate([
        _wchunk(inputs["W_state"], NFP8, WSCALE).reshape(128, -1),
        _wchunk(inputs["Wcq1"], NFP8, WSCALE).reshape(128, -1),
        _wchunk(inputs["Wcq2"], NFP8, WSCALE).reshape(128, -1)], axis=1)
    cf_tail = np.concatenate([
        _rsb(inputs["b_state"], 2, WSCALE),
        _rsb(inputs["bcq1"], 4, WSCALE),
        _rsb(inputs["bcq2"], 4, WSCALE),
        _rsb(inputs["bq"], 32, WSCALE * QCS),
        _rsb(inputs["bagg"], 4),
        _rsb(inputs["brk1"], 4),
        _rsb(inputs["brv1"], 4)], axis=1).astype(np.float32)
    shared = {
        "Wq": _wchunk(inputs["Wq"], NFP8, WSCALE).reshape(128, 2, 2, H * KD),
        "Wagg": _wchunk(inputs["Wagg"]),
        "Wrk1": _wchunk(inputs["Wrk1"]),
        "Wrk2": _wchunk(inputs["Wrk2"]),
        "brk2_flat": np.ascontiguousarray(f(inputs["brk2"])[None, :]),
        "Wrv1": _wchunk(inputs["Wrv1"]),
        "Wrv2": _wchunk(inputs["Wrv2"]),
        "brv2_flat": np.ascontiguousarray(f(inputs["brv2"])[None, :]),
    }
    kfold = keys * rpe * (KSCALE * RSQK)            # [L, 64, K]
    in_maps = []
    for m in range(NCORES):
        envs = [int(order[8 * s + m]) for s in range(BL)]
        kp = np.zeros((128, 2, 2, NCH * 128), NFP8)
        vp = np.zeros((128, NCH, VD), NBF16)
        rowbias = np.zeros((128, NCH), np.float32)
        for p, s in enumerate(SEQ):
            e = envs[s]
            nl = cstar[s] * 128
            c0, c1 = int(offs[p]), int(offs[p + 1])
            kb = kfold[:nl, e, :].T.reshape(2, 2, 128, nl).transpose(
                2, 0, 1, 3)
            kp[:, :, :, c0 * 128:c1 * 128] = kb.astype(NFP8)
            vb = vals[:nl, e, :].reshape(cstar[s], 128, VD).transpose(1, 0, 2)
            vp[:, c0:c1, :] = vb.astype(NBF16)
            labs = (np.arange(128)[:, None]
                    + 128 * np.arange(c1 - c0)[None, :])
            rowbias[:, c0:c1] = np.where(labs < int(step[e]), 0.0, -1e30)
        a8 = np.concatenate([
            _actT(state[envs], NFP8).reshape(128, -1),
            _actT(lat[envs], NFP8).reshape(128, -1),
            A8w], axis=1)
        cf = np.concatenate([rowbias, cf_tail], axis=1).astype(np.float32)
        in_maps.append({
            "keysT": kp, "vals": vp, "A8": np.ascontiguousarray(a8),
            "CF": np.ascontiguousarray(cf),
            **shared,
        })
    return in_maps, order


def kernel(**inputs):
    order, cstar = _plan(inputs["step"])
    nc = _CACHE.get(cstar)
    if nc is None:
        nc = _CACHE[cstar] = _build(cstar)
    in_maps, order = _shard(inputs)
    res = run_bass_kernel_spmd(nc, in_maps, list(range(NCORES)),
                               **_CACHE.get("run_kwargs", {}))
    _CACHE["last_result"] = res
    ok = np.empty((B, RIMQ), np.float32)
    ov = np.empty((B, VD), np.float32)
    for m in range(NCORES):
        for s in range(BL):
            e = int(order[8 * s + m])
            ok[e] = res.results[m]["out_key"][s]
            ov[e] = res.results[m]["out_val"][s]
    return ok[:, None, :], ov[:, None, :]


# revision 43
# speedup vs baseline: 1.1755x; 1.0409x over previous
"""DND retrieval (episodic memory read) kernel for 8 Trainium2 NeuronCores.

Data-parallel over batch B=64 -> 8 envs per core, with step-aware
packing: only ceil(step/128) l-chunks per env are ever touched (the
rest are masked to zero by the softmax validity mask), so the host
packs exactly those chunks, assigns envs to cores by sorted rank so
every core shares one compiled chunk pattern C*, and the kernel skips
the dead ~45% of keys/vals DMA and PE work.

Precision: keys (with rpe * 64/sqrt(K) folded in) and the q-side MLP
stream as fp8e4m3 (weights x32, qc x32, q x16 host/chip scales); the
scores and Wq matmuls run in fp8 DoubleRow mode (2 contraction rows
per partition, 2x PE rate). vals and output-side weights stay bf16
(fp8 there pushes error past budget).

Scores are processed in 512-column windows of the packed image through
a 2-bank PSUM ring: scores -> exp(S/1024) -> multiply by a precomputed
validity mask -> unnormalized probs transpose straight into the value
matmul; softmax 1/Z is applied to the [64, 512] result instead
(linearity), so nothing waits on the global sum. Scores are tiny
(|s| < 0.3), so no max pass is needed.
"""
from contextlib import ExitStack

import numpy as np
import ml_dtypes

import concourse.bass as bass
import concourse.tile as tile
from concourse import bacc, mybir
from concourse.bass_utils import run_bass_kernel_spmd
from concourse.masks import make_identity

F32 = mybir.dt.float32
BF16 = mybir.dt.bfloat16
FP8 = mybir.dt.float8e4
AF = mybir.ActivationFunctionType
OP = mybir.AluOpType
DR = mybir.MatmulPerfMode.DoubleRow

L = 1024
B = 64        # rows of the batched softmax image: (slot, head)
BL = 8        # envs (slots) per core
KD = 512
VD = 512
H = 8
MEMB = 256
SDIM = 512
HID = 512
RIMQ = 512
LAT = KD - MEMB
NCORES = 8
KC = KD // 128
RSQK = 1.0 / np.sqrt(np.float32(KD))
KSCALE = 64.0          # folded into keys on host
WSCALE = 32.0          # fp8 weight scale
QCS = 32.0             # qc activation fp8 scale
QS = 16.0              # q fp8 scale inside Qpad
NBF16 = np.dtype(ml_dtypes.bfloat16)
NFP8 = np.dtype(ml_dtypes.float8_e4m3)
SEQ = [0, 7, 1, 6, 2, 5, 3, 4]   # packed slot order

_CACHE: dict = {}


def _emit(nc: bass.Bass, tc: tile.TileContext, ctx: ExitStack, io: dict,
          cstar: tuple):
    # ---- packed geometry (compile-time) ----
    seqc = [cstar[s] for s in SEQ]
    offs = np.concatenate([[0], np.cumsum(seqc)])
    NCH = int(offs[-1])
    W = NCH * 128
    owner = []                       # chunk idx -> slot
    for p, s in enumerate(SEQ):
        owner += [s] * seqc[p]
    NW = (NCH + 3) // 4              # 512-col score windows
    NS = (NW + 1) // 2               # keys DMA slabs (2 windows each)

    pool = ctx.enter_context(tc.tile_pool(name="main", bufs=1))
    kpool = ctx.enter_context(tc.tile_pool(name="keys", bufs=2 * NS))
    wpool = ctx.enter_context(tc.tile_pool(name="wstream", bufs=2))
    psum = ctx.enter_context(tc.tile_pool(name="ps", bufs=4, space="PSUM"))
    spsum = ctx.enter_context(tc.tile_pool(name="ps2", bufs=2, space="PSUM"))
    rpsum = ctx.enter_context(tc.tile_pool(name="ps3", bufs=1, space="PSUM"))

    identb = pool.tile([128, 128], BF16)
    make_identity(nc, identb[:])

    def bias_tile(name, nch, eng=None):
        t = pool.tile([128, nch], F32, tag="b" + name)
        (eng or nc.sync).dma_start(t[:], io[name][:])
        return t

    # One fp8 blob (5 KB lines) carries all phase-A operands: the former
    # per-tensor DMAs had 8-140 B partition lines whose descriptor overhead
    # stalled the sync queue ~25 us before W_state even started.
    A8 = pool.tile([128, 5168], FP8)
    nc.sync.dma_start(A8[:], io["A8"][:])
    CF = pool.tile([128, NCH + 54], F32)
    nc.sync.dma_start(CF[:], io["CF"][:])
    rowbias = CF[:, 0:NCH]
    bst = CF[:, NCH:NCH + 2]
    bcq1 = CF[:, NCH + 2:NCH + 6]
    bcq2 = CF[:, NCH + 6:NCH + 10]
    bq = CF[:, NCH + 10:NCH + 42]

    # ---------------- Phase A: q-side MLP (fp8, DoubleRow Wq) -------------
    stateT_n = A8[:, 0:32].rearrange("p (k b) -> p k b", k=4)
    latT_n = A8[:, 32:48].rearrange("p (k b) -> p k b", k=2)
    w_state = A8[:, 48:1072].rearrange("p (k c) -> p k c", k=4)
    w_cq1 = A8[:, 1072:3120].rearrange("p (k c) -> p k c", k=4)
    w_cq2 = A8[:, 3120:5168].rearrange("p (k c) -> p k c", k=4)

    stateT = [stateT_n[:, c, :] for c in range(SDIM // 128)]
    latT = [latT_n[:, c, :] for c in range(LAT // 128)]

    def layer_T(xT_chunks, w, b_tile, n_out, tag, scale=None):
        nk = len(xT_chunks)
        outs = []
        for j in range(n_out // 128):
            ps = psum.tile([128, BL], F32, tag="sm")
            for k in range(nk):
                nc.tensor.matmul(ps[:], w[:, k, j * 128:(j + 1) * 128],
                                 xT_chunks[k], start=(k == 0),
                                 stop=(k == nk - 1), skip_group_check=True)
            t = pool.tile([128, BL], BF16, tag=f"{tag}{j}")
            if scale is None:
                nc.vector.tensor_scalar(out=t[:], in0=ps[:],
                                        scalar1=b_tile[:, j:j + 1],
                                        scalar2=None, op0=OP.add)
            else:
                nc.vector.tensor_scalar(out=t[:], in0=ps[:],
                                        scalar1=b_tile[:, j:j + 1],
                                        scalar2=scale, op0=OP.add,
                                        op1=OP.mult)
            outs.append(t[:])
        return outs

    def layer_Tio(xT_chunks, w_name, b_tile, n_out, tag, eng=None):
        nk = len(xT_chunks)
        w = wpool.tile([128, nk, n_out], BF16, tag="Wstgb")
        (eng or nc.sync).dma_start(w[:], io[w_name][:])
        return layer_T(xT_chunks, w[:], b_tile, n_out, tag)

    RW = 1.0 / WSCALE
    xT = layer_T(stateT, w_state, bst, MEMB, "xT", scale=RW) + latT
    h1T = layer_T(xT, w_cq1, bcq1, HID, "h1", scale=RW)
    # qc layer -> single fp8 tile (x QCS), consumed as DoubleRow lhsT.
    # Padded to QCW columns: dual-fp8 LDWEIGHTS rejects 8-wide loads.
    QCW = 32
    qcT = pool.tile([128, KC, QCW], FP8)
    nc.gpsimd.memset(qcT[:], 0.0)
    for j in range(KC):
        ps = psum.tile([128, BL], F32, tag="sm")
        for k in range(KC):
            nc.tensor.matmul(ps[:], w_cq2[:, k, j * 128:(j + 1) * 128],
                             h1T[k], start=(k == 0), stop=(k == KC - 1),
                             skip_group_check=True)
        nc.vector.tensor_scalar(out=qcT[:, j, 0:BL], in0=ps[:],
                                scalar1=bcq2[:, j:j + 1], scalar2=QCS / 32.0,
                                op0=OP.add, op1=OP.mult)

    # Wq in DoubleRow fp8: out [8, 512] per (jg, kcp), then transpose and
    # scatter into Qpad (fp8, xQS) diagonal windows.
    Qpad = pool.tile([128, 2, 2, BL, 72], FP8)
    nc.gpsimd.memset(Qpad[:], 0.0)
    wq = pool.tile([128, 2, 2, H * KD], FP8)
    for kcp in range(2):
        (nc.sync if kcp == 0 else nc.scalar).dma_start(
            wq[:, kcp, :, :], io["Wq"][:, kcp, :, :])
    QSC = QS / (32.0 * QCS)
    for jg in range(8):
        ps = spsum.tile([QCW, 512], F32, tag="sp")
        for kcp in range(2):
            nc.tensor.matmul(ps[:], qcT[:, 2 * kcp:2 * kcp + 2, :],
                             wq[:, kcp, :, jg * 512:(jg + 1) * 512],
                             start=(kcp == 0), stop=(kcp == 1),
                             perf_mode=DR, skip_group_check=True)
        qsb = pool.tile([BL, 512], BF16, tag="qsb")
        nc.scalar.copy(qsb[:], ps[0:BL, :])
        for jj in range(4):
            j = jg * 4 + jj
            h, kc = j // KC, j % KC
            tp = psum.tile([128, BL], BF16, tag="sm")
            nc.tensor.transpose(tp[:], qsb[:, jj * 128:(jj + 1) * 128],
                                identb[0:BL, 0:BL])
            nc.vector.tensor_scalar(
                out=Qpad[:, kc // 2, kc % 2, :, h], in0=tp[:],
                scalar1=bq[:, j:j + 1], scalar2=QSC, op0=OP.add, op1=OP.mult)

    # ------- keys first (all slabs), then vals; queues never block keys ---
    slabs = []
    vres = pool.tile([128, NCH, VD], BF16)
    for si in range(NS):
        c0, c1 = 8 * si, min(8 * si + 8, NCH)
        kts = []
        for kcp in range(2):
            kt = kpool.tile([128, 2, 1024], FP8, tag="kt")
            (nc.sync if kcp == 0 else nc.scalar).dma_start(
                kt[:, :, 0:(c1 - c0) * 128],
                io["keysT"][:, kcp, :, c0 * 128:c1 * 128])
            kts.append(kt)
        slabs.append(kts)
    vengs = [nc.gpsimd, nc.scalar, nc.sync]
    for si in range(NS):
        c0, c1 = 8 * si, min(8 * si + 8, NCH)
        vengs[si % 3].dma_start(vres[:, c0:c1, :], io["vals"][:, c0:c1, :])
    wagg = pool.tile([128, 32, VD], BF16)
    waeng = [nc.gpsimd, nc.sync, nc.gpsimd, nc.sync]
    for gi in range(4):
        waeng[gi].dma_start(wagg[:, gi * 8:(gi + 1) * 8, :],
                            io["Wagg"][:, gi * 8:(gi + 1) * 8, :])

    # ------- per-chunk: scoresT -> exp(+bias) -> value matmul + Z ----------
    # scoresT [128(l), 8(h)] per chunk; exp writes masked unnormalized probs
    # straight into the transposed EVT image the value matmul consumes.
    EVT = pool.tile([128, NCH, B], BF16)
    nc.gpsimd.memset(EVT[:], 0.0)
    onesb = pool.tile([128, 1], BF16)
    nc.gpsimd.memset(onesb[:], 1.0)
    rps = rpsum.tile([B, VD], F32, tag="rp")
    zps = rpsum.tile([B, 1], F32, tag="z")

    def chunkwork(i):
        nc.tensor.matmul(rps[:], EVT[:, i, :], vres[:, i, :],
                         start=(i == 0), stop=(i == NCH - 1),
                         skip_group_check=True)
        nc.tensor.matmul(zps[:], EVT[:, i, :], onesb[:],
                         start=(i == 0), stop=(i == NCH - 1),
                         skip_group_check=True)

    for i in range(NCH):
        s = owner[i]
        si, sc0 = i // 8, (i % 8) * 128
        kts = slabs[si]
        sgt = psum.tile([128, H], F32, tag="sm")
        for kcp in range(2):
            nc.tensor.matmul(sgt[:], kts[kcp][:, :, sc0:sc0 + 128],
                             Qpad[:, kcp, :, s, 0:H],
                             start=(kcp == 0), stop=(kcp == 1),
                             perf_mode=DR, skip_group_check=True)
        nc.scalar.activation(EVT[:, i, s * H:(s + 1) * H], sgt[:], AF.Exp,
                             bias=rowbias[:, i:i + 1],
                             scale=1.0 / (KSCALE * QS))
        if i > 5:
            chunkwork(i - 6)
    for i in range(max(NCH - 6, 0), NCH):
        chunkwork(i)

    # R = 1/Z folded into the result readout
    R = pool.tile([B, 1], F32)
    nc.vector.reciprocal(R[:], zps[:])
    rsb = pool.tile([B, VD], BF16, tag="rs")
    nc.vector.tensor_scalar(out=rsb[:], in0=rps[:], scalar1=R[:, 0:1],
                            scalar2=None, op0=OP.mult)
    RT = pool.tile([128, VD // 128, B], BF16)
    for vc in range(VD // 128):
        tr = psum.tile([128, B], BF16, tag="sm")
        nc.tensor.transpose(tr[:], rsb[:, vc * 128:(vc + 1) * 128],
                            identb[0:B, 0:B])
        nc.vector.tensor_copy(RT[:, vc, :], tr[:])

    # ---------------- Phase E: output MLP chain (bf16) ---------------------
    bagg = CF[:, NCH + 42:NCH + 46]
    brk1 = CF[:, NCH + 46:NCH + 50]
    brv1 = CF[:, NCH + 50:NCH + 54]

    aggp = spsum.tile([BL, VD], F32, tag="sp")
    for c in range(32):
        h, vc = c // 4, c % 4
        nc.tensor.matmul(aggp[:], RT[:, vc, h:B:H], wagg[:, c, :],
                         start=(c == 0), stop=(c == 31),
                         skip_group_check=True)
    aggsb = pool.tile([BL, VD], BF16, tag="aggsb")
    nc.scalar.copy(aggsb[:], aggp[:])
    AT = []
    for j in range(VD // 128):
        tp = psum.tile([128, BL], BF16, tag="sm")
        nc.tensor.transpose(tp[:], aggsb[:, j * 128:(j + 1) * 128],
                            identb[0:BL, 0:BL])
        t = pool.tile([128, BL], BF16, tag=f"AT{j}")
        nc.vector.tensor_scalar(out=t[:], in0=tp[:],
                                scalar1=bagg[:, j:j + 1],
                                scalar2=None, op0=OP.add)
        AT.append(t[:])

    ones = pool.tile([1, BL], F32)
    nc.gpsimd.memset(ones[:], 1.0)

    def bias_bcast(name, eng=None):
        brow = pool.tile([1, 512], F32, tag="br" + name)
        (eng or nc.sync).dma_start(brow[:], io[name][:])
        bb = spsum.tile([BL, 512], F32, tag="sp")
        nc.tensor.matmul(bb[:], ones[:], brow[:], start=True, stop=True)
        bsb = pool.tile([BL, 512], F32, tag="bs" + name)
        nc.vector.tensor_copy(bsb[:], bb[:])
        return bsb

    bk2 = bias_bcast("brk2_flat")
    bv2 = bias_bcast("brv2_flat", eng=nc.scalar)

    def layer_nat(xT_chunks, w_name, n_out, eng=None):
        nk = len(xT_chunks)
        w = wpool.tile([128, nk, n_out], BF16, tag="Wstgb")
        (eng or nc.sync).dma_start(w[:], io[w_name][:])
        ps = spsum.tile([BL, n_out], F32, tag="sp")
        for k in range(nk):
            nc.tensor.matmul(ps[:], xT_chunks[k], w[:, k, :],
                             start=(k == 0), stop=(k == nk - 1),
                             skip_group_check=True)
        return ps

    hkT = layer_Tio(AT, "Wrk1", brk1, HID, "hk")
    ok_ps = layer_nat(hkT, "Wrk2", RIMQ)
    hvT = layer_Tio(AT, "Wrv1", brv1, HID, "hv", eng=nc.scalar)
    ov_ps = layer_nat(hvT, "Wrv2", VD, eng=nc.scalar)

    for name, ps_, bias_sb in (("out_key", ok_ps, bk2), ("out_val", ov_ps, bv2)):
        onat = pool.tile([BL, 512], F32, tag="o" + name)
        nc.vector.tensor_tensor(out=onat[:], in0=ps_[:], in1=bias_sb[:],
                                op=OP.add)
        nc.sync.dma_start(io[name][:], onat[:])


def _build(cstar):
    seqc = [cstar[s] for s in SEQ]
    NCH = int(sum(seqc))
    W = NCH * 128
    NW = (NCH + 3) // 4
    nc = bacc.Bacc("TRN2", target_bir_lowering=False, debug=False,
                   num_devices=NCORES)
    io = {}

    def din(name, shape, dt=BF16):
        io[name] = nc.dram_tensor(name, shape, dt, kind="ExternalInput").ap()

    din("keysT", [128, 2, 2, W], FP8)
    din("vals", [128, NCH, VD])
    din("A8", [128, 5168], FP8)
    din("CF", [128, NCH + 54], F32)
    din("Wq", [128, 2, 2, H * KD], FP8)
    din("Wagg", [128, 32, VD])
    din("Wrk1", [128, KC, HID])
    din("Wrk2", [128, KC, RIMQ])
    din("brk2_flat", [1, 512], F32)
    din("Wrv1", [128, KC, HID])
    din("Wrv2", [128, KC, VD])
    din("brv2_flat", [1, 512], F32)
    io["out_key"] = nc.dram_tensor("out_key", [BL, RIMQ], F32,
                                   kind="ExternalOutput").ap()
    io["out_val"] = nc.dram_tensor("out_val", [BL, VD], F32,
                                   kind="ExternalOutput").ap()

    with tile.TileContext(nc) as tc, ExitStack() as ctx:
        _emit(nc, tc, ctx, io, cstar)
    nc.compile()
    return nc


def _rsb(bias, nch, scale=1.0):
    return np.ascontiguousarray(
        np.asarray(bias, np.float32).reshape(nch, 128).T * scale)


def _wchunk(w, dt=NBF16, scale=1.0):
    w = np.asarray(w, np.float32) * scale
    f, c = w.shape
    return np.ascontiguousarray(
        w.reshape(f // 128, 128, c).transpose(1, 0, 2)).astype(dt)


def _actT(x, dt):
    x = np.asarray(x, np.float32)
    bl, f = x.shape
    return np.ascontiguousarray(
        x.T.reshape(f // 128, 128, bl).transpose(1, 0, 2)).astype(dt)


def _plan(step):
    cb = np.clip((np.asarray(step, np.int64) + 127) // 128, 1, 8)
    order = np.argsort(-cb, kind="stable")
    cstar = tuple(int(cb[order[8 * s]]) for s in range(BL))
    return order, cstar


def _shard(inputs):
    f = lambda x: np.asarray(x, np.float32)
    keys, vals, rpe = f(inputs["keys"]), f(inputs["vals"]), f(inputs["rpe_mod"])
    step = np.asarray(inputs["step"]).astype(np.int64)
    state, lat = f(inputs["state"]), f(inputs["task_inference_latent"])

    order, cstar = _plan(step)
    seqc = [cstar[s] for s in SEQ]
    offs = np.concatenate([[0], np.cumsum(seqc)])
    NCH = int(offs[-1])
    NW = (NCH + 3) // 4

    A8w = np.concatenate([
        _wchunk(inputs["W_state"], NFP8, WSCALE).reshape(128, -1),
        _wchunk(inputs["Wcq1"], NFP8, WSCALE).reshape(128, -1),
        _wchunk(inputs["Wcq2"], NFP8, WSCALE).reshape(128, -1)], axis=1)
    cf_tail = np.concatenate([
        _rsb(inputs["b_state"], 2, WSCALE),
        _rsb(inputs["bcq1"], 4, WSCALE),
        _rsb(inputs["bcq2"], 4, WSCALE),
        _rsb(inputs["bq"], 32, WSCALE * QCS),
        _rsb(inputs["bagg"], 4),
        _rsb(inputs["brk1"], 4),
        _rsb(inputs["brv1"], 4)], axis=1).astype(np.float32)
    shared = {
        "Wq": _wchunk(inputs["Wq"], NFP8, WSCALE).reshape(128, 2, 2, H * KD),
        "Wagg": _wchunk(inputs["Wagg"]),
        "Wrk1": _wchunk(inputs["Wrk1"]),
        "Wrk2": _wchunk(inputs["Wrk2"]),
        "brk2_flat": np.ascontiguousarray(f(inputs["brk2"])[None, :]),
        "Wrv1": _wchunk(inputs["Wrv1"]),
        "Wrv2": _wchunk(inputs["Wrv2"]),
        "brv2_flat": np.ascontiguousarray(f(inputs["brv2"])[None, :]),
    }
    kfold = keys * rpe * (KSCALE * RSQK)            # [L, 64, K]
    in_maps = []
    for m in range(NCORES):
        envs = [int(order[8 * s + m]) for s in range(BL)]
        kp = np.zeros((128, 2, 2, NCH * 128), NFP8)
        vp = np.zeros((128, NCH, VD), NBF16)
        rowbias = np.zeros((128, NCH), np.float32)
        for p, s in enumerate(SEQ):
            e = envs[s]
            nl = cstar[s] * 128
            c0, c1 = int(offs[p]), int(offs[p + 1])
            kb = kfold[:nl, e, :].T.reshape(2, 2, 128, nl).transpose(
                2, 0, 1, 3)
            kp[:, :, :, c0 * 128:c1 * 128] = kb.astype(NFP8)
            vb = vals[:nl, e, :].reshape(cstar[s], 128, VD).transpose(1, 0, 2)
            vp[:, c0:c1, :] = vb.astype(NBF16)
            labs = (np.arange(128)[:, None]
                    + 128 * np.arange(c1 - c0)[None, :])
            rowbias[:, c0:c1] = np.where(labs < int(step[e]), 0.0, -1e30)
        a8 = np.concatenate([
            _actT(state[envs], NFP8).reshape(128, -1),
            _actT(lat[envs], NFP8).reshape(128, -1),
            A8w], axis=1)
        cf = np.concatenate([rowbias, cf_tail], axis=1).astype(np.float32)
        in_maps.append({
            "keysT": kp, "vals": vp, "A8": np.ascontiguousarray(a8),
            "CF": np.ascontiguousarray(cf),
            **shared,
        })
    return in_maps, order


def kernel(**inputs):
    order, cstar = _plan(inputs["step"])
    nc = _CACHE.get(cstar)
    if nc is None:
        nc = _CACHE[cstar] = _build(cstar)
    in_maps, order = _shard(inputs)
    res = run_bass_kernel_spmd(nc, in_maps, list(range(NCORES)),
                               **_CACHE.get("run_kwargs", {}))
    _CACHE["last_result"] = res
    ok = np.empty((B, RIMQ), np.float32)
    ov = np.empty((B, VD), np.float32)
    for m in range(NCORES):
        for s in range(BL):
            e = int(order[8 * s + m])
            ok[e] = res.results[m]["out_key"][s]
            ov[e] = res.results[m]["out_val"][s]
    return ok[:, None, :], ov[:, None, :]
